# revision 1
# baseline (speedup 1.0000x reference)
"""Trainium2 Bass kernel for nn_AttnLayer (dense_transformer, sum-normalized attention).

Reference computation (per batch b, all fp32):
    d      = in_seq[:,b,:] @ W_in2enc.T + prev_target_seq[:,b,:] @ W_lab2enc.T + (b1+b2)
    S      = d @ E.T                      (E = enc_seq[:,b,:])
    ssum_l = sum_m S[l,m]                 (sum-normalization denominator)
    C      = S @ E
    out    = (C / ssum[:,None]) @ W_enc2in.T + b3

Mapping (PE computes out = lhsT.T @ rhs, contraction over the partition dim).
The attention is linear (sum-normalization, no softmax), so the score matrix S
is never materialized: C = S @ E = d @ (E^T E) = d @ G with the 512x512 Gram
matrix G — halving the score/context FLOPs:
    G    = E-blocks.T @ E                            [e, e']  float32r
    H    = G-blocks.T @ W3T                          [e, o]   float32r  (= G @ W3T)
    d^T  = W1T.T @ X^T + W2T.T @ T^T + bd           [e, l]   float32r
    O    = d^T-blocks.T @ H                          [l, o]   float32r, then *1/ssum + b3
    ssum = X @ v1 + T @ v2 + esum.bd  with v1 = W1.T@esum, esum = sum_m E[m,:]
           (v1/v2 via fp32 PE matmuls against a partition-replicated esum, so the
            result lands pre-broadcast; the X/T matvecs run as gpsimd elementwise
            products + DVE row-reductions over naturally-laid-out X/T tiles, with
            per-chunk reciprocals so output scaling starts early.)

Numerics: ssum suffers catastrophic cancellation (min |ssum| ~ 0.05 vs ~700
typical) and 1/ssum scales the whole row, so the entire ssum path is native
fp32 (PE fp32 verified 1.4e-7, DVE is exact fp32). The big matmuls use
float32r (~14-bit mantissa, HW-verified 1.4e-4 on K=512 dots) whose errors
stay *relative* to the numerator and cancel against the same 1/ssum.

Sharding: data-parallel over batch B=16 across 8 cores (2 batches per core).
Host-side glue pre-transposes per-core slices so every DMA is contiguous.
"""

import os

os.environ.setdefault("MYCRO_LOCAL_CACHE", "1")

import numpy as np

import concourse.bass as bass
from concourse import bacc
import concourse.mybir as mybir
import concourse.tile as tile
from concourse.bass_utils import run_bass_kernel_spmd

# Problem shape (hardcoded per contract)
L = 1024      # L_in == L_enc
B = 16
D = 512       # D_in == D_enc == D_emb
N_CORES = 8
BPC = B // N_CORES   # batches per core
P = 128
NE = D // P          # 4 chunks of contraction axes of size 512
NM = L // P          # 8 chunks of the L_enc axis
NL = L // P          # 8 chunks of the L_in axis
NLH = 2              # l processed in halves of 512 (moving-operand max for 4-byte)
LH = L // NLH

F32 = mybir.dt.float32
F32R = mybir.dt.float32r


def build_nc():
    nc = bacc.Bacc(None, target_bir_lowering=False, debug=False)

    xT_d = nc.declare_dram_parameter("xT", [BPC, D, L], F32, isOutput=False)
    tT_d = nc.declare_dram_parameter("tT", [BPC, D, L], F32, isOutput=False)
    eT_d = nc.declare_dram_parameter("eT", [BPC, D, L], F32, isOutput=False)
    en_d = nc.declare_dram_parameter("en", [BPC, L, D], F32, isOutput=False)
    xn_d = nc.declare_dram_parameter("xn", [BPC, L, D], F32, isOutput=False)
    tn_d = nc.declare_dram_parameter("tn", [BPC, L, D], F32, isOutput=False)
    w1t_d = nc.declare_dram_parameter("w1t", [D, D], F32, isOutput=False)  # [i, e]
    w2t_d = nc.declare_dram_parameter("w2t", [D, D], F32, isOutput=False)  # [j, e]
    w3t_d = nc.declare_dram_parameter("w3t", [D, D], F32, isOutput=False)  # [e, o]
    w1n_d = nc.declare_dram_parameter("w1n", [D, D], F32, isOutput=False)  # [e, i]
    w2n_d = nc.declare_dram_parameter("w2n", [D, D], F32, isOutput=False)  # [e, j]
    bd_d = nc.declare_dram_parameter("bd", [D], F32, isOutput=False)       # b1 + b2
    b3bc_d = nc.declare_dram_parameter("b3bc", [P, D], F32, isOutput=False)
    out_d = nc.declare_dram_parameter("out", [BPC, L, D], F32, isOutput=True)

    AF = mybir.ActivationFunctionType
    AX = mybir.AxisListType
    ALU = mybir.AluOpType

    with tile.TileContext(nc) as tc:
        with (
            tc.tile_pool(name="wpool", bufs=1) as wpool,
            tc.tile_pool(name="big", bufs=1) as big,
            tc.tile_pool(name="vecs", bufs=1) as vecs,
            tc.tile_pool(name="opool", bufs=10) as opool,
            tc.tile_pool(name="psA", bufs=4, space="PSUM") as psA,
            tc.tile_pool(name="psCol", bufs=1, space="PSUM") as psCol,
        ):
            # DMA queues: SP(sync) = Xnat/Tnat + output stores; ACT(scalar) = eT
            # chunks + fp32 weights; SWDGE(gpsimd) = all f32r cast loads.
            # Program order favors the startup critical path.
            w1t = wpool.tile([P, NE, D], F32R, name="w1t")
            w2t = wpool.tile([P, NE, D], F32R, name="w2t")
            w3t = wpool.tile([P, NE, D], F32R, name="w3t")
            w1n = wpool.tile([P, NE, D], F32, name="w1n")
            w2n = wpool.tile([P, NE, D], F32, name="w2n")
            bd_sb = wpool.tile([P, NE], F32, name="bd_sb")
            b3bc = wpool.tile([P, D], F32, name="b3bc")

            def emit_O(bb, dT_b, H_b, rcols_b, lcs):
                # O[l,o] = d^T-blocks.T @ H, * (1/ssum), + b3
                for lc in lcs:
                    o_ps = psA.tile([P, D], F32, name="o_ps", tag="oacc", bufs=3)
                    for ec in range(NE):
                        nc.tensor.matmul(
                            o_ps,
                            dT_b[:, ec, lc * P : (lc + 1) * P],
                            H_b[:, ec, :],
                            start=(ec == 0), stop=(ec == NE - 1),
                        )
                    o_sb = opool.tile([P, D], F32, name="o_sb")
                    nc.scalar.activation(
                        o_sb, o_ps, AF.Copy, bias=0.0,
                        scale=rcols_b[:, lc : lc + 1])
                    nc.vector.tensor_add(o_sb, o_sb, b3bc)
                    nc.sync.dma_start(
                        out=out_d[bb, lc * P : (lc + 1) * P, :], in_=o_sb)

            stash = []
            for b in range(BPC):
                # ---- per-batch loads, chunked so consumers start early ----
                # Slot map (same tag = same SBUF memory, recycled by lifetime):
                #   slot_A: eT (dies early) -> dT       slot_B: Xnat
                #   slot_C: tT_r (dies at step1) -> sT  slot_D: xT_r -> cT
                #   slot_E: Tnat   slot_F: eT_r   slot_I: en_r
                xn = big.tile([P, NL, D], F32, name="xn", tag="slot_B")
                tn = big.tile([P, NL, D], F32, name="tn", tag="slot_E")
                eT = big.tile([P, NE, L], F32, name="eT", tag="slot_A")
                en_r = big.tile([P, NM, D], F32R, name="en_r", tag="slot_I")
                xT_r = big.tile([P, NE, L], F32R, name="xT_r", tag="slot_D")
                tT_r = big.tile([P, NE, L], F32R, name="tT_r", tag="slot_C")

                # en_r first: the Gram matmuls are PE's startup work and
                # need only en_r chunks. Then step1-critical loads, finely
                # chunked: the first (lh0, ec0) group needs w1t e-slice 0 +
                # xT_r k-chunk 0.
                for mc in range(NM):
                    nc.gpsimd.dma_start(
                        out=en_r[:, mc, :], in_=en_d[b, mc * P : (mc + 1) * P, :])
                for ec in range(NE):
                    nc.gpsimd.dma_start(
                        out=eT[:, ec, :], in_=eT_d[b, ec * P : (ec + 1) * P, :])
                for k in range(NE):
                    rows = slice(k * P, (k + 1) * P)
                    if b == 0:
                        nc.gpsimd.dma_start(
                            out=w1t[:, :, k * P : (k + 1) * P],
                            in_=w1t_d[:, k * P : (k + 1) * P]
                            .rearrange("(c p) e -> p c e", p=P))
                        nc.gpsimd.dma_start(
                            out=w2t[:, :, k * P : (k + 1) * P],
                            in_=w2t_d[:, k * P : (k + 1) * P]
                            .rearrange("(c p) e -> p c e", p=P))
                    nc.gpsimd.dma_start(out=xT_r[:, k, :], in_=xT_d[b, rows, :])
                    nc.gpsimd.dma_start(out=tT_r[:, k, :], in_=tT_d[b, rows, :])
                if b == 0:
                    nc.scalar.dma_start(
                        out=w1n, in_=w1n_d.rearrange("(c p) e -> p c e", p=P))
                    nc.scalar.dma_start(
                        out=w2n, in_=w2n_d.rearrange("(c p) e -> p c e", p=P))
                    nc.scalar.dma_start(
                        out=bd_sb, in_=bd_d.rearrange("(c p) -> p c", p=P))
                nc.scalar.dma_start(out=xn, in_=xn_d[b].rearrange("(c p) e -> p c e", p=P))
                nc.scalar.dma_start(out=tn, in_=tn_d[b].rearrange("(c p) e -> p c e", p=P))
                if b == 0:
                    nc.gpsimd.dma_start(
                        out=w3t, in_=w3t_d.rearrange("(c p) e -> p c e", p=P))
                    nc.scalar.dma_start(out=b3bc, in_=b3bc_d[:, :])



                # ---- G[e,e'] = E.T @ E (Gram, f32r) — PE's startup work ----
                G_sb = big.tile([P, NE, D], F32R, name="G_sb", tag="slot_G")
                for gc in range(NE):
                    g_ps = psA.tile([P, D], F32, name="g_ps", tag="acc")
                    for mc in range(NM):
                        nc.tensor.matmul(
                            g_ps,
                            en_r[:, mc, gc * P : (gc + 1) * P],
                            en_r[:, mc, :],
                            start=(mc == 0), stop=(mc == NM - 1),
                        )
                    nc.scalar.activation(G_sb[:, gc, :], g_ps, AF.Copy, bias=0.0)

                if b == 1:
                    emit_O(0, *stash[0], range(NL // 2, NL))

                # ---- esum[e] = sum_m E[m, e]; replicate across partitions ----
                esum = vecs.tile([P, NE], F32, name="esum")
                esumB = vecs.tile([P, NE, P], F32, name="esumB")
                for ec in range(NE):
                    nc.vector.reduce_sum(esum[:, ec : ec + 1], eT[:, ec, :], axis=AX.X)
                    nc.vector.tensor_copy(
                        esumB[:, ec, :], esum[:, ec : ec + 1].broadcast_to([P, P]))

                # ---- v1 = W1.T@esum, v2 = W2.T@esum, c0 = esum.bd (fp32 PE, all
                # outputs partition-replicated because lhsT columns are equal) ----
                v1b_ps = psA.tile([P, D], F32, name="v1b_ps", tag="acc")
                v2b_ps = psA.tile([P, D], F32, name="v2b_ps", tag="acc")
                c0b_ps = psCol.tile([P, 1], F32, name="c0b_ps", tag="colvec")
                for ec in range(NE):
                    nc.tensor.matmul(
                        v1b_ps, esumB[:, ec, :], w1n[:, ec, :],
                        start=(ec == 0), stop=(ec == NE - 1))
                for ec in range(NE):
                    nc.tensor.matmul(
                        v2b_ps, esumB[:, ec, :], w2n[:, ec, :],
                        start=(ec == 0), stop=(ec == NE - 1))
                for ec in range(NE):
                    nc.tensor.matmul(
                        c0b_ps, esumB[:, ec, :], bd_sb[:, ec : ec + 1],
                        start=(ec == 0), stop=(ec == NE - 1))
                v1b = vecs.tile([P, D], F32, name="v1b")
                v2b = vecs.tile([P, D], F32, name="v2b")
                c0b = vecs.tile([P, 1], F32, name="c0b")
                nc.vector.tensor_copy(v1b, v1b_ps)
                nc.vector.tensor_copy(v2b, v2b_ps)
                nc.vector.tensor_copy(c0b, c0b_ps)


                # ---- H[e,o] = G @ W3T  (f32r) ----
                H_sb = big.tile([P, NE, D], F32R, name="H_sb", tag="slot_H")
                for hc in range(NE):
                    h_ps = psA.tile([P, D], F32, name="h_ps", tag="acc")
                    for kc in range(NE):
                        nc.tensor.matmul(
                            h_ps,
                            G_sb[:, kc, hc * P : (hc + 1) * P],
                            w3t[:, kc, :],
                            start=(kc == 0), stop=(kc == NE - 1),
                        )
                    nc.vector.tensor_copy(H_sb[:, hc, :], h_ps)

                # ---- d^T[e,l] = W1T.T @ X^T + W2T.T @ T^T + bd  (f32r) ----
                dT = big.tile([P, NE, L], F32R, name="dT", tag="slot_F")
                K_ORDER = [0, 1, 4, 5, 2, 3, 6, 7]  # first input halves first
                for lh in range(NLH):
                    for ec in range(NE):
                        d_ps = psA.tile([P, LH], F32, name="d_ps", tag="acc")
                        for i, k in enumerate(K_ORDER):
                            w = w1t if k < NE else w2t
                            src = xT_r if k < NE else tT_r
                            nc.tensor.matmul(
                                d_ps,
                                w[:, k % NE, ec * P : (ec + 1) * P],
                                src[:, k % NE, lh * LH : (lh + 1) * LH],
                                start=(i == 0), stop=(i == 2 * NE - 1),
                            )
                        nc.vector.tensor_scalar_add(
                            dT[:, ec, lh * LH : (lh + 1) * LH], d_ps,
                            bd_sb[:, ec : ec + 1],
                        )




                # ---- ssum[l] = X@v1 + T@v2 + c0, exact fp32, in l-chunk
                # column layout: gpsimd multiplies (free-dim-broadcast v),
                # DVE reduces over the 512-wide feature axis ----
                rx = vecs.tile([P, NL], F32, name="rx")
                rt = vecs.tile([P, NL], F32, name="rt")
                sc_sb = vecs.tile([P, NL], F32, name="sc_sb")
                # scrA shares SBUF with G_sb: G dies (H built) before ssum runs
                scrA = big.tile([P, 2, D], F32, name="scrA", tag="slot_G")
                scrB = vecs.tile([P, 2, D], F32, name="scrB")
                rcols = vecs.tile([P, NL], F32, name="rcols")
                for q in range(NL // 2):
                    cs = slice(q * 2, q * 2 + 2)
                    nc.gpsimd.tensor_mul(
                        scrA, xn[:, cs, :],
                        v1b[:, None, :].broadcast_to([P, 2, D]))
                    nc.vector.reduce_sum(rx[:, cs], scrA, axis=AX.X)
                    nc.gpsimd.tensor_mul(
                        scrB, tn[:, cs, :],
                        v2b[:, None, :].broadcast_to([P, 2, D]))
                    nc.vector.reduce_sum(rt[:, cs], scrB, axis=AX.X)
                    # incremental: O(lc) needs only its own rcols column
                    nc.vector.tensor_add(sc_sb[:, cs], rx[:, cs], rt[:, cs])
                    nc.vector.tensor_scalar_add(sc_sb[:, cs], sc_sb[:, cs], c0b)
                    nc.vector.reciprocal(rcols[:, cs], sc_sb[:, cs])

                # ---- output stage, software-pipelined across batches:
                # first half now (early rcols chunks), second half emitted
                # after the next batch's Gram work so PE is never starved
                # while the remaining 1/ssum columns trickle in ----
                if b == 0 and BPC > 1:
                    emit_O(b, dT, H_sb, rcols, range(NL // 2))
                    stash.append((dT, H_sb, rcols))
                else:
                    emit_O(b, dT, H_sb, rcols, range(NL))

    nc.finalize()
    return nc


def _make_in_maps(in_seq, enc_seq, prev_target_seq, W_in2enc, b_in2enc,
                  W_lab2enc, b_lab2enc, W_enc2in, b_enc2in):
    f32 = np.float32
    w1t = np.ascontiguousarray(np.asarray(W_in2enc, f32).T)   # [i, e]
    w2t = np.ascontiguousarray(np.asarray(W_lab2enc, f32).T)  # [j, e]
    w3t = np.ascontiguousarray(np.asarray(W_enc2in, f32).T)   # [e, o]
    w1n = np.ascontiguousarray(np.asarray(W_in2enc, f32))
    w2n = np.ascontiguousarray(np.asarray(W_lab2enc, f32))
    bd = np.ascontiguousarray(np.asarray(b_in2enc, f32) + np.asarray(b_lab2enc, f32))
    b3bc = np.ascontiguousarray(np.broadcast_to(np.asarray(b_enc2in, f32), (P, D)))

    in_maps = []
    for c in range(N_CORES):
        bs = slice(c * BPC, (c + 1) * BPC)
        x = np.asarray(in_seq[:, bs, :], f32)
        t = np.asarray(prev_target_seq[:, bs, :], f32)
        e = np.asarray(enc_seq[:, bs, :], f32)
        in_maps.append({
            "xT": np.ascontiguousarray(x.transpose(1, 2, 0)),
            "tT": np.ascontiguousarray(t.transpose(1, 2, 0)),
            "eT": np.ascontiguousarray(e.transpose(1, 2, 0)),
            "en": np.ascontiguousarray(e.transpose(1, 0, 2)),
            "xn": np.ascontiguousarray(x.transpose(1, 0, 2)),
            "tn": np.ascontiguousarray(t.transpose(1, 0, 2)),
            "w1t": w1t, "w2t": w2t, "w3t": w3t, "w1n": w1n, "w2n": w2n,
            "bd": bd, "b3bc": b3bc,
        })
    return in_maps


_NC_CACHE = {}


def _get_nc():
    if "nc" not in _NC_CACHE:
        _NC_CACHE["nc"] = build_nc()
    return _NC_CACHE["nc"]


def kernel(**inputs):
    in_maps = _make_in_maps(**inputs)
    nc = _get_nc()
    res = run_bass_kernel_spmd(nc, in_maps, core_ids=list(range(N_CORES)))
    out = np.empty((L, B, D), np.float32)
    for c in range(N_CORES):
        per_core = res.results[c]["out"]  # (BPC, L, D)
        for j in range(BPC):
            out[:, c * BPC + j, :] = per_core[j]
    return out


def kernel_sim(core_id=0, **inputs):
    """CoreSim validation path: simulate one core, return its (BPC, L, D) output."""
    from concourse.bass_interp import CoreSim

    in_maps = _make_in_maps(**inputs)
    nc = _get_nc()
    sim = CoreSim(nc)
    for name, val in in_maps[core_id].items():
        sim.tensor(name)[:] = val
    sim.simulate(check_with_hw=False)
    return np.array(sim.tensor("out"))



# revision 12
# speedup vs baseline: 1.4041x; 1.4041x over previous
"""Trainium2 Bass kernel for nn_AttnLayer (dense_transformer, sum-normalized attention).

Reference computation (per batch b, all fp32):
    d      = in_seq[:,b,:] @ W_in2enc.T + prev_target_seq[:,b,:] @ W_lab2enc.T + (b1+b2)
    S      = d @ E.T                      (E = enc_seq[:,b,:])
    ssum_l = sum_m S[l,m]                 (sum-normalization denominator)
    C      = S @ E
    out    = (C / ssum[:,None]) @ W_enc2in.T + b3

The attention is linear (sum-normalization, no softmax), so S is never
materialized: C = d @ (E^T E) = d @ G with the 512x512 Gram matrix G:
    G    = E-blocks.T @ E                            [e, e']  f32r
    H    = G-blocks.T @ W3T                          [e, o]   f32r  (= G @ W3T)
    d^T  = W1T.T @ X^T + W2T.T @ T^T + bd            [e, l]   f32r
    O    = d^T-blocks.T @ H                          [l, o]   f32r, then *1/ssum + b3

Denominator (exact fp32; ssum suffers catastrophic cancellation, min |ssum|
~0.05 vs ~700 typical, so this path must be fp32 end-to-end from raw inputs):
    ssum = X@v1 + T@v2 + esum.bd,  v1 = W1^T esum,  esum = sum_m E[m,:]
Everything on this path runs as matmuls whose MOVING operand is a [K,1]
column, which costs ~nothing on the PE (cost ~ moving rows only; stationary
loads are free):
  - esum[e]  = ones-column reduction of E chunks over partitions
  - v1,v2    = W^T @ esum-column (accumulated over e-chunks)
  - ssum[l]  = X-chunk.T @ v1-column (accumulated over chunks of both inputs)
HW-verified: fp32 ap-size-1 matmuls are exact-fp32-class; f32r data is
rounded to an 11-bit mantissa AT LOAD (DMA) or at engine write, NOT at PE
read.  So the fp32 inputs are loaded once exactly, the denominator reads
them directly, and cheap engine round-copies produce the f32r versions the
numerator matmuls consume (errors there stay relative to the numerator and
cancel against 1/ssum).

Sharding: data-parallel over batch B=16 across 8 cores (2 batches per core).
"""

import os

os.environ.setdefault("MYCRO_LOCAL_CACHE", "1")

import numpy as np

import concourse.bass as bass
from concourse import bacc
import concourse.mybir as mybir
import concourse.tile as tile
from concourse.bass_utils import run_bass_kernel_spmd

# Problem shape (hardcoded per contract)
L = 1024      # L_in == L_enc
B = 16
D = 512       # D_in == D_enc == D_emb
N_CORES = 8
BPC = B // N_CORES   # batches per core
P = 128
NE = D // P          # 4 chunks of contraction axes of size 512
NM = L // P          # 8 chunks of the L_enc axis
NL = L // P          # 8 chunks of the L_in axis
NLH = 2              # l processed in halves of 512 (moving-operand max for 4-byte)
LH = L // NLH

F32 = mybir.dt.float32
F32R = mybir.dt.float32r

# smallT PSUM column map: [0:32) esum partials (mc,ec), [32:36) v1,
# [36:40) v2, [40] c0, [41:49) ssum
C_ESUM = 0
C_V1 = 32
C_V2 = 36
C_C0 = 40
C_SS = 41


def build_nc():
    nc = bacc.Bacc(None, target_bir_lowering=False, debug=False)

    xT_d = nc.declare_dram_parameter("xT", [BPC, D, L], F32, isOutput=False)
    tT_d = nc.declare_dram_parameter("tT", [BPC, D, L], F32, isOutput=False)
    en_d = nc.declare_dram_parameter("en", [BPC, L, D], F32, isOutput=False)
    w1t_d = nc.declare_dram_parameter("w1t", [D, D], F32R, isOutput=False)  # [i, e]
    w2t_d = nc.declare_dram_parameter("w2t", [D, D], F32R, isOutput=False)  # [j, e]
    w3t_d = nc.declare_dram_parameter("w3t", [D, D], F32R, isOutput=False)  # [e, o]
    w1n_d = nc.declare_dram_parameter("w1n", [D, D], F32, isOutput=False)  # [e, i]
    w2n_d = nc.declare_dram_parameter("w2n", [D, D], F32, isOutput=False)  # [e, j]
    bd_d = nc.declare_dram_parameter("bd", [D], F32, isOutput=False)       # b1 + b2
    b3bc_d = nc.declare_dram_parameter("b3bc", [P, D], F32, isOutput=False)
    ones_d = nc.declare_dram_parameter("ones", [P, 1], F32, isOutput=False)
    out_d = nc.declare_dram_parameter("out", [BPC, L, D], F32, isOutput=True)

    AF = mybir.ActivationFunctionType
    AX = mybir.AxisListType

    with tile.TileContext(nc) as tc:
        with (
            tc.tile_pool(name="wpool", bufs=1) as wpool,
            tc.tile_pool(name="big", bufs=1) as big,
            tc.tile_pool(name="vecs", bufs=1) as vecs,
            tc.tile_pool(name="opool", bufs=10) as opool,
            tc.tile_pool(name="psA", bufs=4, space="PSUM") as psA,
            tc.tile_pool(name="psS", bufs=1, space="PSUM") as psS,
        ):
            # ---- persistent weights / constants ----
            w1t = wpool.tile([P, NE, D], F32R, name="w1t")   # [i%128, i//128, e]
            w2t = wpool.tile([P, NE, D], F32R, name="w2t")
            w3t = wpool.tile([P, NE, D], F32R, name="w3t")   # [e%128, e//128, o]
            w1n = wpool.tile([P, NE, D], F32, name="w1n")    # [e%128, e//128, i]
            w2n = wpool.tile([P, NE, D], F32, name="w2n")
            bd_sb = wpool.tile([P, NE], F32, name="bd_sb")
            b3bc = wpool.tile([P, D], F32, name="b3bc")
            ones = wpool.tile([P, 1], F32, name="ones")

            # Small constants on the ACT queue (tiny transfers).
            nc.scalar.dma_start(out=ones, in_=ones_d[:, :])
            nc.scalar.dma_start(
                out=bd_sb, in_=bd_d.rearrange("(c p) -> p c", p=P))
            nc.scalar.dma_start(out=b3bc, in_=b3bc_d[:, :])

            # ---- the single ordered bulk-load queue (SP): need order ----
            en = [None, None]
            xT = [None, None]
            tT = [None, None]
            en[0] = big.tile([P, NM, D], F32, name="en0", tag="slot_en")
            xT[0] = big.tile([P, NE, L], F32, name="xT0", tag="slot_x")
            tT[0] = big.tile([P, NE, L], F32, name="tT0", tag="slot_t")
            for mc in range(NM):
                nc.sync.dma_start(
                    out=en[0][:, mc, :], in_=en_d[0, mc * P : (mc + 1) * P, :])
            nc.sync.dma_start(
                out=w1n, in_=w1n_d.rearrange("(c p) e -> p c e", p=P))
            nc.sync.dma_start(
                out=w2n, in_=w2n_d.rearrange("(c p) e -> p c e", p=P))
            for k in range(NE):
                nc.sync.dma_start(
                    out=w1t[:, k, :], in_=w1t_d[k * P : (k + 1) * P, :])
                nc.sync.dma_start(
                    out=w2t[:, k, :], in_=w2t_d[k * P : (k + 1) * P, :])
            for k in range(NE):
                nc.sync.dma_start(out=xT[0][:, k, :], in_=xT_d[0, k * P : (k + 1) * P, :])
                nc.sync.dma_start(out=tT[0][:, k, :], in_=tT_d[0, k * P : (k + 1) * P, :])
            nc.sync.dma_start(
                out=w3t, in_=w3t_d.rearrange("(c p) e -> p c e", p=P))
            en[1] = big.tile([P, NM, D], F32, name="en1", tag="slot_en")
            for mc in range(NM):
                nc.sync.dma_start(
                    out=en[1][:, mc, :], in_=en_d[1, mc * P : (mc + 1) * P, :])
            xT[1] = big.tile([P, NE, L], F32, name="xT1", tag="slot_x")
            tT[1] = big.tile([P, NE, L], F32, name="tT1", tag="slot_t")
            for k in range(NE):
                nc.sync.dma_start(out=xT[1][:, k, :], in_=xT_d[1, k * P : (k + 1) * P, :])
                nc.sync.dma_start(out=tT[1][:, k, :], in_=tT_d[1, k * P : (k + 1) * P, :])

            # round-copy engine rotation (writes F32R <- reads F32, rounds)
            rr_eng = [nc.gpsimd, nc.vector, nc.scalar]

            def rcopy(i, out, in_):
                eng = rr_eng[i % 3]
                if eng is nc.scalar:
                    eng.activation(out, in_, AF.Copy, bias=0.0)
                else:
                    eng.tensor_copy(out, in_)

            for b in range(BPC):
                enb, xTb, tTb = en[b], xT[b], tT[b]

                smallT = psS.tile([P, C_SS + NL], F32, name="smallT", tag="smallps")

                # ---- per-chunk: esum partials (exact fp32 ap1s), then engine
                # round-copy to en_r for the Gram path ----
                en_r = big.tile([P, NM, D], F32R, name="en_r", tag="slot_enr")
                for mc in range(NM):
                    for ec in range(NE):
                        nc.tensor.matmul(
                            smallT[:, C_ESUM + 4 * mc + ec : C_ESUM + 4 * mc + ec + 1],
                            enb[:, mc, ec * P : (ec + 1) * P],
                            ones,
                            start=True, stop=True,
                            skip_group_check=True,
                        )
                    rcopy(mc, en_r[:, mc, :], enb[:, mc, :])

                # ---- Gram G[e,e'] = E.T @ E (f32r on rounded copy) ----
                G_sb = big.tile([P, NE, D], F32R, name="G_sb", tag="slot_G")
                for gc in range(NE):
                    g_ps = psA.tile([P, D], F32, name="g_ps", tag="acc")
                    for mc in range(NM):
                        nc.tensor.matmul(
                            g_ps,
                            en_r[:, mc, gc * P : (gc + 1) * P],
                            en_r[:, mc, :],
                            start=(mc == 0), stop=(mc == NM - 1),
                        )
                    nc.scalar.activation(G_sb[:, gc, :], g_ps, AF.Copy, bias=0.0)

                # esum consolidation on DVE: esum_sb[:, ec] = sum_mc partial
                esum_sb = vecs.tile([P, NE], F32, name="esum_sb")
                nc.vector.tensor_copy(esum_sb, smallT[:, C_ESUM : C_ESUM + 4])
                for mc in range(1, NM):
                    nc.vector.tensor_add(
                        esum_sb, esum_sb,
                        smallT[:, C_ESUM + 4 * mc : C_ESUM + 4 * mc + 4])
                # c0 partial: t[p] = sum_c bd[c,p]*esum[c,p]; bcast along free
                c0t = vecs.tile([P, 1], F32, name="c0t")
                c0m = vecs.tile([P, NE], F32, name="c0m")
                c0B = vecs.tile([P, P], F32, name="c0B")
                nc.vector.tensor_mul(c0m, bd_sb, esum_sb)
                nc.vector.reduce_sum(c0t, c0m, axis=AX.X)
                nc.vector.tensor_copy(c0B, c0t.broadcast_to([P, P]))

                # ---- v1 = W1^T esum, v2 = W2^T esum, c0 (ap1 fp32) ----
                for ic in range(NE):
                    for ec in range(NE):
                        nc.tensor.matmul(
                            smallT[:, C_V1 + ic : C_V1 + ic + 1],
                            w1n[:, ec, ic * P : (ic + 1) * P],
                            esum_sb[:, ec : ec + 1],
                            start=(ec == 0), stop=(ec == NE - 1),
                            skip_group_check=True,
                        )
                for ic in range(NE):
                    for ec in range(NE):
                        nc.tensor.matmul(
                            smallT[:, C_V2 + ic : C_V2 + ic + 1],
                            w2n[:, ec, ic * P : (ic + 1) * P],
                            esum_sb[:, ec : ec + 1],
                            start=(ec == 0), stop=(ec == NE - 1),
                            skip_group_check=True,
                        )
                nc.tensor.matmul(
                    smallT[:, C_C0 : C_C0 + 1], c0B, ones, start=True, stop=True,
                    skip_group_check=True,
                )
                v1c = vecs.tile([P, NE], F32, name="v1c")
                v2c = vecs.tile([P, NE], F32, name="v2c")
                c0c = vecs.tile([P, 1], F32, name="c0c")
                nc.scalar.activation(v1c, smallT[:, C_V1 : C_V1 + NE], AF.Copy, bias=0.0)
                nc.scalar.activation(v2c, smallT[:, C_V2 : C_V2 + NE], AF.Copy, bias=0.0)
                nc.scalar.activation(c0c, smallT[:, C_C0 : C_C0 + 1], AF.Copy, bias=0.0)

                # round-copies of X^T / T^T for the f32r dT matmuls; the exact
                # fp32 tiles keep serving the denominator below
                xT_r = big.tile([P, NE, L], F32R, name="xT_r", tag="slot_xr")
                tT_r = big.tile([P, NE, L], F32R, name="tT_r", tag="slot_tr")
                for k in range(NE):
                    rcopy(k, xT_r[:, k, :], xTb[:, k, :])
                    rcopy(k + 1, tT_r[:, k, :], tTb[:, k, :])

                # ---- d^T[e,l] = W1T.T @ X^T + W2T.T @ T^T + bd (f32r) ----
                dT = big.tile([P, NE, L], F32R, name="dT", tag="slot_d")
                for lh in range(NLH):
                    for ec in range(NE):
                        d_ps = psA.tile([P, LH], F32, name="d_ps", tag="acc")
                        for k in range(2 * NE):
                            w = w1t if k < NE else w2t
                            src = xT_r if k < NE else tT_r
                            nc.tensor.matmul(
                                d_ps,
                                w[:, k % NE, ec * P : (ec + 1) * P],
                                src[:, k % NE, lh * LH : (lh + 1) * LH],
                                start=(k == 0), stop=(k == 2 * NE - 1),
                            )
                        nc.vector.tensor_scalar_add(
                            dT[:, ec, lh * LH : (lh + 1) * LH], d_ps,
                            bd_sb[:, ec : ec + 1],
                        )

                # ---- ssum[l] = X@v1 + T@v2 directly as ap1 fp32 matmuls:
                # stationary = fp32 data chunk, moving = v column ----
                sc_sb = vecs.tile([P, NL], F32, name="sc_sb")
                rcols = vecs.tile([P, NL], F32, name="rcols")
                for lc in range(NL):
                    for k in range(2 * NE):
                        data = xTb if k < NE else tTb
                        vcol = v1c if k < NE else v2c
                        nc.tensor.matmul(
                            smallT[:, C_SS + lc : C_SS + lc + 1],
                            data[:, k % NE, lc * P : (lc + 1) * P],
                            vcol[:, k % NE : k % NE + 1],
                            start=(k == 0), stop=(k == 2 * NE - 1),
                            skip_group_check=True,
                        )
                for lc in range(NL):
                    nc.vector.tensor_scalar_add(
                        sc_sb[:, lc : lc + 1], smallT[:, C_SS + lc : C_SS + lc + 1],
                        c0c)
                    nc.vector.reciprocal(
                        rcols[:, lc : lc + 1], sc_sb[:, lc : lc + 1])

                # ---- H[e,o] = G @ W3T (f32r); rcols compute on DVE meanwhile ----
                H_sb = big.tile([P, NE, D], F32R, name="H_sb", tag="slot_H")
                for hc in range(NE):
                    h_ps = psA.tile([P, D], F32, name="h_ps", tag="acc")
                    for kc in range(NE):
                        nc.tensor.matmul(
                            h_ps,
                            G_sb[:, kc, hc * P : (hc + 1) * P],
                            w3t[:, kc, :],
                            start=(kc == 0), stop=(kc == NE - 1),
                        )
                    nc.scalar.activation(H_sb[:, hc, :], h_ps, AF.Copy, bias=0.0)

                # ---- output stage: O[l,o] = dT.T @ H, * 1/ssum, + b3 ----
                for lc in range(NL):
                    o_ps = psA.tile([P, D], F32, name="o_ps", tag="oacc", bufs=3)
                    for ec in range(NE):
                        nc.tensor.matmul(
                            o_ps,
                            dT[:, ec, lc * P : (lc + 1) * P],
                            H_sb[:, ec, :],
                            start=(ec == 0), stop=(ec == NE - 1),
                        )
                    o_sb = opool.tile([P, D], F32, name="o_sb")
                    nc.scalar.activation(
                        o_sb, o_ps, AF.Copy, bias=0.0,
                        scale=rcols[:, lc : lc + 1])
                    nc.vector.tensor_add(o_sb, o_sb, b3bc)
                    nc.scalar.dma_start(
                        out=out_d[b, lc * P : (lc + 1) * P, :], in_=o_sb)

    nc.finalize()
    return nc


def _make_in_maps(in_seq, enc_seq, prev_target_seq, W_in2enc, b_in2enc,
                  W_lab2enc, b_lab2enc, W_enc2in, b_enc2in):
    f32 = np.float32
    w1t = np.ascontiguousarray(np.asarray(W_in2enc, f32).T)   # [i, e]
    w2t = np.ascontiguousarray(np.asarray(W_lab2enc, f32).T)  # [j, e]
    w3t = np.ascontiguousarray(np.asarray(W_enc2in, f32).T)   # [e, o]
    w1n = np.ascontiguousarray(np.asarray(W_in2enc, f32))
    w2n = np.ascontiguousarray(np.asarray(W_lab2enc, f32))
    bd = np.ascontiguousarray(np.asarray(b_in2enc, f32) + np.asarray(b_lab2enc, f32))
    b3bc = np.ascontiguousarray(np.broadcast_to(np.asarray(b_enc2in, f32), (P, D)))
    ones = np.ones((P, 1), f32)

    in_maps = []
    for c in range(N_CORES):
        bs = slice(c * BPC, (c + 1) * BPC)
        x = np.asarray(in_seq[:, bs, :], f32)
        t = np.asarray(prev_target_seq[:, bs, :], f32)
        e = np.asarray(enc_seq[:, bs, :], f32)
        in_maps.append({
            "xT": np.ascontiguousarray(x.transpose(1, 2, 0)),
            "tT": np.ascontiguousarray(t.transpose(1, 2, 0)),
            "en": np.ascontiguousarray(e.transpose(1, 0, 2)),
            "w1t": w1t, "w2t": w2t, "w3t": w3t, "w1n": w1n, "w2n": w2n,
            "bd": bd, "b3bc": b3bc, "ones": ones,
        })
    return in_maps


_NC_CACHE = {}


def _get_nc():
    if "nc" not in _NC_CACHE:
        _NC_CACHE["nc"] = build_nc()
    return _NC_CACHE["nc"]


def kernel(**inputs):
    in_maps = _make_in_maps(**inputs)
    nc = _get_nc()
    res = run_bass_kernel_spmd(nc, in_maps, core_ids=list(range(N_CORES)))
    out = np.empty((L, B, D), np.float32)
    for c in range(N_CORES):
        per_core = res.results[c]["out"]  # (BPC, L, D)
        for j in range(BPC):
            out[:, c * BPC + j, :] = per_core[j]
    return out


def kernel_sim(core_id=0, **inputs):
    """CoreSim validation path: simulate one core, return its (BPC, L, D) output."""
    from concourse.bass_interp import CoreSim

    in_maps = _make_in_maps(**inputs)
    nc = _get_nc()
    sim = CoreSim(nc)
    for name, val in in_maps[core_id].items():
        sim.tensor(name)[:] = val
    sim.simulate(check_with_hw=False)
    return np.array(sim.tensor("out"))


# revision 13
# speedup vs baseline: 1.5279x; 1.0882x over previous
"""Trainium2 Bass kernel for nn_AttnLayer (dense_transformer, sum-normalized attention).

Reference computation (per batch b, all fp32):
    d      = in_seq[:,b,:] @ W_in2enc.T + prev_target_seq[:,b,:] @ W_lab2enc.T + (b1+b2)
    S      = d @ E.T                      (E = enc_seq[:,b,:])
    ssum_l = sum_m S[l,m]                 (sum-normalization denominator)
    C      = S @ E
    out    = (C / ssum[:,None]) @ W_enc2in.T + b3

The attention is linear (sum-normalization, no softmax), so S is never
materialized: C = d @ (E^T E) = d @ G with the 512x512 Gram matrix G:
    G    = E-blocks.T @ E                            [e, e']  f32r
    H    = G-blocks.T @ W3T                          [e, o]   f32r  (= G @ W3T)
    d^T  = W1T.T @ X^T + W2T.T @ T^T + bd            [e, l]   f32r
    O    = d^T-blocks.T @ H                          [l, o]   f32r, then *1/ssum + b3

Denominator (exact fp32; ssum suffers catastrophic cancellation, min |ssum|
~0.05 vs ~700 typical, so this path must be fp32 end-to-end from raw inputs):
    ssum = X@v1 + T@v2 + esum.bd,  v1 = W1^T esum,  esum = sum_m E[m,:]
Everything on this path runs as matmuls whose MOVING operand is a [K,1]
column, which costs ~nothing on the PE (cost ~ moving rows only; stationary
loads are free):
  - esum[e]  = ones-column reduction of E chunks over partitions
  - v1,v2    = W^T @ esum-column (accumulated over e-chunks)
  - ssum[l]  = X-chunk.T @ v1-column (accumulated over chunks of both inputs)
HW-verified: fp32 ap-size-1 matmuls are exact-fp32-class; f32r data is
rounded to an 11-bit mantissa AT LOAD (DMA) or at engine write, NOT at PE
read.  So the fp32 inputs are loaded once exactly, the denominator reads
them directly, and cheap engine round-copies produce the f32r versions the
numerator matmuls consume (errors there stay relative to the numerator and
cancel against 1/ssum).

Sharding: data-parallel over batch B=16 across 8 cores (2 batches per core).
"""

import os

os.environ.setdefault("MYCRO_LOCAL_CACHE", "1")

import numpy as np

import concourse.bass as bass
from concourse import bacc
import concourse.mybir as mybir
import concourse.tile as tile
from concourse.bass_utils import run_bass_kernel_spmd

# Problem shape (hardcoded per contract)
L = 1024      # L_in == L_enc
B = 16
D = 512       # D_in == D_enc == D_emb
N_CORES = 8
BPC = B // N_CORES   # batches per core
P = 128
NE = D // P          # 4 chunks of contraction axes of size 512
NM = L // P          # 8 chunks of the L_enc axis
NL = L // P          # 8 chunks of the L_in axis
NLH = 2              # l processed in halves of 512 (moving-operand max for 4-byte)
LH = L // NLH

F32 = mybir.dt.float32
F32R = mybir.dt.float32r

# smallT PSUM column map: [0:32) esum partials (mc,ec), [32:36) v1,
# [36:40) v2, [40] c0, [41:49) ssum
C_ESUM = 0
C_V1 = 32
C_V2 = 36
C_C0 = 40
C_SS = 41


def build_nc():
    nc = bacc.Bacc(None, target_bir_lowering=False, debug=False)

    xT_d = nc.declare_dram_parameter("xT", [BPC, D, L], F32, isOutput=False)
    tT_d = nc.declare_dram_parameter("tT", [BPC, D, L], F32, isOutput=False)
    en_d = nc.declare_dram_parameter("en", [BPC, L, D], F32, isOutput=False)
    w1t_d = nc.declare_dram_parameter("w1t", [D, D], F32R, isOutput=False)  # [i, e]
    w2t_d = nc.declare_dram_parameter("w2t", [D, D], F32R, isOutput=False)  # [j, e]
    w3t_d = nc.declare_dram_parameter("w3t", [D, D], F32R, isOutput=False)  # [e, o]
    w1n_d = nc.declare_dram_parameter("w1n", [D, D], F32, isOutput=False)  # [e, i]
    w2n_d = nc.declare_dram_parameter("w2n", [D, D], F32, isOutput=False)  # [e, j]
    bd_d = nc.declare_dram_parameter("bd", [D], F32, isOutput=False)       # b1 + b2
    b3bc_d = nc.declare_dram_parameter("b3bc", [P, D], F32, isOutput=False)
    ones_d = nc.declare_dram_parameter("ones", [P, 1], F32, isOutput=False)
    out_d = nc.declare_dram_parameter("out", [BPC, L, D], F32, isOutput=True)

    AF = mybir.ActivationFunctionType
    AX = mybir.AxisListType

    with tile.TileContext(nc) as tc:
        with (
            tc.tile_pool(name="wpool", bufs=1) as wpool,
            tc.tile_pool(name="big", bufs=1) as big,
            tc.tile_pool(name="vecs", bufs=1) as vecs,
            tc.tile_pool(name="opool", bufs=10) as opool,
            tc.tile_pool(name="psA", bufs=4, space="PSUM") as psA,
            tc.tile_pool(name="psS", bufs=1, space="PSUM") as psS,
        ):
            # ---- persistent weights / constants ----
            w1t = wpool.tile([P, NE, D], F32R, name="w1t")   # [i%128, i//128, e]
            w2t = wpool.tile([P, NE, D], F32R, name="w2t")
            w3t = wpool.tile([P, NE, D], F32R, name="w3t")   # [e%128, e//128, o]
            w1n = wpool.tile([P, NE, D], F32, name="w1n")    # [e%128, e//128, i]
            w2n = wpool.tile([P, NE, D], F32, name="w2n")
            bd_sb = wpool.tile([P, NE], F32, name="bd_sb")
            b3bc = wpool.tile([P, D], F32, name="b3bc")
            ones = wpool.tile([P, 1], F32, name="ones")

            # Small constants on the ACT queue (tiny transfers).
            nc.scalar.dma_start(out=ones, in_=ones_d[:, :])
            nc.scalar.dma_start(
                out=bd_sb, in_=bd_d.rearrange("(c p) -> p c", p=P))
            nc.scalar.dma_start(out=b3bc, in_=b3bc_d[:, :])

            # ---- the single ordered bulk-load queue (SP): need order ----
            en = [None, None]
            xT = [None, None]
            tT = [None, None]
            en[0] = big.tile([P, NM, D], F32, name="en0", tag="slot_en")
            xT[0] = big.tile([P, NE, L], F32, name="xT0", tag="slot_x")
            tT[0] = big.tile([P, NE, L], F32, name="tT0", tag="slot_t")
            for mc in range(NM):
                nc.sync.dma_start(
                    out=en[0][:, mc, :], in_=en_d[0, mc * P : (mc + 1) * P, :])
            for k in range(NE):
                nc.sync.dma_start(
                    out=w1t[:, k, :], in_=w1t_d[k * P : (k + 1) * P, :])
                nc.sync.dma_start(
                    out=w2t[:, k, :], in_=w2t_d[k * P : (k + 1) * P, :])
                nc.sync.dma_start(out=xT[0][:, k, :], in_=xT_d[0, k * P : (k + 1) * P, :])
                nc.sync.dma_start(out=tT[0][:, k, :], in_=tT_d[0, k * P : (k + 1) * P, :])
            nc.sync.dma_start(
                out=w1n, in_=w1n_d.rearrange("(c p) e -> p c e", p=P))
            nc.sync.dma_start(
                out=w2n, in_=w2n_d.rearrange("(c p) e -> p c e", p=P))
            nc.sync.dma_start(
                out=w3t, in_=w3t_d.rearrange("(c p) e -> p c e", p=P))
            en[1] = big.tile([P, NM, D], F32, name="en1", tag="slot_en")
            for mc in range(NM):
                nc.sync.dma_start(
                    out=en[1][:, mc, :], in_=en_d[1, mc * P : (mc + 1) * P, :])
            xT[1] = big.tile([P, NE, L], F32, name="xT1", tag="slot_x")
            tT[1] = big.tile([P, NE, L], F32, name="tT1", tag="slot_t")
            for k in range(NE):
                nc.sync.dma_start(out=xT[1][:, k, :], in_=xT_d[1, k * P : (k + 1) * P, :])
                nc.sync.dma_start(out=tT[1][:, k, :], in_=tT_d[1, k * P : (k + 1) * P, :])

            # round-copy engine rotation (writes F32R <- reads F32, rounds)
            rr_eng = [nc.gpsimd, nc.vector, nc.scalar]

            def rcopy(i, out, in_):
                eng = rr_eng[i % 3]
                if eng is nc.scalar:
                    eng.activation(out, in_, AF.Copy, bias=0.0)
                else:
                    eng.tensor_copy(out, in_)

            for b in range(BPC):
                enb, xTb, tTb = en[b], xT[b], tT[b]

                smallT = psS.tile([P, C_SS + NL], F32, name="smallT", tag="smallps")

                # ---- per-chunk: esum partials (exact fp32 ap1s, single-
                # matmul groups), engine round-copy to en_r, then the chunk's
                # 4 Gram matmuls (mc-outer, 4-bank interleaved accumulation)
                # so PE consumes chunks at DMA-arrival rate ----
                en_r = big.tile([P, NM, D], F32R, name="en_r", tag="slot_enr")
                G_sb = big.tile([P, NE, D], F32R, name="G_sb", tag="slot_G")
                g_ps = [psA.tile([P, D], F32, name=f"g_ps{gc}", tag="acc")
                        for gc in range(NE)]
                for mc in range(NM):
                    for ec in range(NE):
                        nc.tensor.matmul(
                            smallT[:, C_ESUM + 4 * mc + ec : C_ESUM + 4 * mc + ec + 1],
                            enb[:, mc, ec * P : (ec + 1) * P],
                            ones,
                            start=True, stop=True,
                            skip_group_check=True,
                        )
                    rcopy(mc, en_r[:, mc, :], enb[:, mc, :])
                    for gc in range(NE):
                        nc.tensor.matmul(
                            g_ps[gc],
                            en_r[:, mc, gc * P : (gc + 1) * P],
                            en_r[:, mc, :],
                            start=(mc == 0), stop=(mc == NM - 1),
                        )
                for gc in range(NE):
                    nc.scalar.activation(G_sb[:, gc, :], g_ps[gc], AF.Copy, bias=0.0)

                # round-copies of X^T / T^T for the f32r dT matmuls; the exact
                # fp32 tiles keep serving the denominator below
                xT_r = big.tile([P, NE, L], F32R, name="xT_r", tag="slot_xr")
                tT_r = big.tile([P, NE, L], F32R, name="tT_r", tag="slot_tr")
                for k in range(NE):
                    rcopy(2 * k, xT_r[:, k, :], xTb[:, k, :])
                    rcopy(2 * k + 1, tT_r[:, k, :], tTb[:, k, :])

                # ---- d^T[e,l] = W1T.T @ X^T + W2T.T @ T^T + bd (f32r);
                # k-outer in DMA-arrival order (x0,t0,x1,t1,...) across 4
                # interleaved ec banks so PE paces with the loads ----
                dT = big.tile([P, NE, L], F32R, name="dT", tag="slot_d")
                K_ARRIVAL = [0, 4, 1, 5, 2, 6, 3, 7]
                for lh in range(NLH):
                    d_ps = [psA.tile([P, LH], F32, name=f"d_ps{ec}", tag="acc")
                            for ec in range(NE)]
                    for i, k in enumerate(K_ARRIVAL):
                        w = w1t if k < NE else w2t
                        src = xT_r if k < NE else tT_r
                        for ec in range(NE):
                            nc.tensor.matmul(
                                d_ps[ec],
                                w[:, k % NE, ec * P : (ec + 1) * P],
                                src[:, k % NE, lh * LH : (lh + 1) * LH],
                                start=(i == 0), stop=(i == 2 * NE - 1),
                            )
                    for ec in range(NE):
                        nc.vector.tensor_scalar_add(
                            dT[:, ec, lh * LH : (lh + 1) * LH], d_ps[ec],
                            bd_sb[:, ec : ec + 1],
                        )

                # esum consolidation on DVE: esum_sb[:, ec] = sum_mc partial
                esum_sb = vecs.tile([P, NE], F32, name="esum_sb")
                nc.vector.tensor_copy(esum_sb, smallT[:, C_ESUM : C_ESUM + 4])
                for mc in range(1, NM):
                    nc.vector.tensor_add(
                        esum_sb, esum_sb,
                        smallT[:, C_ESUM + 4 * mc : C_ESUM + 4 * mc + 4])
                # c0 partial: t[p] = sum_c bd[c,p]*esum[c,p]; bcast along free
                c0t = vecs.tile([P, 1], F32, name="c0t")
                c0m = vecs.tile([P, NE], F32, name="c0m")
                c0B = vecs.tile([P, P], F32, name="c0B")
                nc.vector.tensor_mul(c0m, bd_sb, esum_sb)
                nc.vector.reduce_sum(c0t, c0m, axis=AX.X)
                nc.vector.tensor_copy(c0B, c0t.broadcast_to([P, P]))

                # ---- v1 = W1^T esum, v2 = W2^T esum, c0 (ap1 fp32) ----
                for ic in range(NE):
                    for ec in range(NE):
                        nc.tensor.matmul(
                            smallT[:, C_V1 + ic : C_V1 + ic + 1],
                            w1n[:, ec, ic * P : (ic + 1) * P],
                            esum_sb[:, ec : ec + 1],
                            start=(ec == 0), stop=(ec == NE - 1),
                            skip_group_check=True,
                        )
                for ic in range(NE):
                    for ec in range(NE):
                        nc.tensor.matmul(
                            smallT[:, C_V2 + ic : C_V2 + ic + 1],
                            w2n[:, ec, ic * P : (ic + 1) * P],
                            esum_sb[:, ec : ec + 1],
                            start=(ec == 0), stop=(ec == NE - 1),
                            skip_group_check=True,
                        )
                nc.tensor.matmul(
                    smallT[:, C_C0 : C_C0 + 1], c0B, ones, start=True, stop=True,
                    skip_group_check=True,
                )
                v1c = vecs.tile([P, NE], F32, name="v1c")
                v2c = vecs.tile([P, NE], F32, name="v2c")
                c0c = vecs.tile([P, 1], F32, name="c0c")
                nc.scalar.activation(v1c, smallT[:, C_V1 : C_V1 + NE], AF.Copy, bias=0.0)
                nc.scalar.activation(v2c, smallT[:, C_V2 : C_V2 + NE], AF.Copy, bias=0.0)
                nc.scalar.activation(c0c, smallT[:, C_C0 : C_C0 + 1], AF.Copy, bias=0.0)

                # ---- ssum[l] = X@v1 + T@v2 directly as ap1 fp32 matmuls:
                # stationary = fp32 data chunk, moving = v column ----
                sc_sb = vecs.tile([P, NL], F32, name="sc_sb")
                rcols = vecs.tile([P, NL], F32, name="rcols")
                for lc in range(NL):
                    for k in range(2 * NE):
                        data = xTb if k < NE else tTb
                        vcol = v1c if k < NE else v2c
                        nc.tensor.matmul(
                            smallT[:, C_SS + lc : C_SS + lc + 1],
                            data[:, k % NE, lc * P : (lc + 1) * P],
                            vcol[:, k % NE : k % NE + 1],
                            start=(k == 0), stop=(k == 2 * NE - 1),
                            skip_group_check=True,
                        )
                for lc in range(NL):
                    nc.vector.tensor_scalar_add(
                        sc_sb[:, lc : lc + 1], smallT[:, C_SS + lc : C_SS + lc + 1],
                        c0c)
                    nc.vector.reciprocal(
                        rcols[:, lc : lc + 1], sc_sb[:, lc : lc + 1])

                # ---- H[e,o] = G @ W3T (f32r); rcols compute on DVE meanwhile ----
                H_sb = big.tile([P, NE, D], F32R, name="H_sb", tag="slot_H")
                for hc in range(NE):
                    h_ps = psA.tile([P, D], F32, name="h_ps", tag="acc")
                    for kc in range(NE):
                        nc.tensor.matmul(
                            h_ps,
                            G_sb[:, kc, hc * P : (hc + 1) * P],
                            w3t[:, kc, :],
                            start=(kc == 0), stop=(kc == NE - 1),
                        )
                    nc.scalar.activation(H_sb[:, hc, :], h_ps, AF.Copy, bias=0.0)

                # ---- output stage: O[l,o] = dT.T @ H, * 1/ssum, + b3 ----
                for lc in range(NL):
                    o_ps = psA.tile([P, D], F32, name="o_ps", tag="oacc", bufs=3)
                    for ec in range(NE):
                        nc.tensor.matmul(
                            o_ps,
                            dT[:, ec, lc * P : (lc + 1) * P],
                            H_sb[:, ec, :],
                            start=(ec == 0), stop=(ec == NE - 1),
                        )
                    o_sb = opool.tile([P, D], F32, name="o_sb")
                    nc.scalar.activation(
                        o_sb, o_ps, AF.Copy, bias=0.0,
                        scale=rcols[:, lc : lc + 1])
                    nc.vector.tensor_add(o_sb, o_sb, b3bc)
                    nc.scalar.dma_start(
                        out=out_d[b, lc * P : (lc + 1) * P, :], in_=o_sb)

    nc.finalize()
    return nc


def _make_in_maps(in_seq, enc_seq, prev_target_seq, W_in2enc, b_in2enc,
                  W_lab2enc, b_lab2enc, W_enc2in, b_enc2in):
    f32 = np.float32
    w1t = np.ascontiguousarray(np.asarray(W_in2enc, f32).T)   # [i, e]
    w2t = np.ascontiguousarray(np.asarray(W_lab2enc, f32).T)  # [j, e]
    w3t = np.ascontiguousarray(np.asarray(W_enc2in, f32).T)   # [e, o]
    w1n = np.ascontiguousarray(np.asarray(W_in2enc, f32))
    w2n = np.ascontiguousarray(np.asarray(W_lab2enc, f32))
    bd = np.ascontiguousarray(np.asarray(b_in2enc, f32) + np.asarray(b_lab2enc, f32))
    b3bc = np.ascontiguousarray(np.broadcast_to(np.asarray(b_enc2in, f32), (P, D)))
    ones = np.ones((P, 1), f32)

    in_maps = []
    for c in range(N_CORES):
        bs = slice(c * BPC, (c + 1) * BPC)
        x = np.asarray(in_seq[:, bs, :], f32)
        t = np.asarray(prev_target_seq[:, bs, :], f32)
        e = np.asarray(enc_seq[:, bs, :], f32)
        in_maps.append({
            "xT": np.ascontiguousarray(x.transpose(1, 2, 0)),
            "tT": np.ascontiguousarray(t.transpose(1, 2, 0)),
            "en": np.ascontiguousarray(e.transpose(1, 0, 2)),
            "w1t": w1t, "w2t": w2t, "w3t": w3t, "w1n": w1n, "w2n": w2n,
            "bd": bd, "b3bc": b3bc, "ones": ones,
        })
    return in_maps


_NC_CACHE = {}


def _get_nc():
    if "nc" not in _NC_CACHE:
        _NC_CACHE["nc"] = build_nc()
    return _NC_CACHE["nc"]


def kernel(**inputs):
    in_maps = _make_in_maps(**inputs)
    nc = _get_nc()
    res = run_bass_kernel_spmd(nc, in_maps, core_ids=list(range(N_CORES)))
    out = np.empty((L, B, D), np.float32)
    for c in range(N_CORES):
        per_core = res.results[c]["out"]  # (BPC, L, D)
        for j in range(BPC):
            out[:, c * BPC + j, :] = per_core[j]
    return out


def kernel_sim(core_id=0, **inputs):
    """CoreSim validation path: simulate one core, return its (BPC, L, D) output."""
    from concourse.bass_interp import CoreSim

    in_maps = _make_in_maps(**inputs)
    nc = _get_nc()
    sim = CoreSim(nc)
    for name, val in in_maps[core_id].items():
        sim.tensor(name)[:] = val
    sim.simulate(check_with_hw=False)
    return np.array(sim.tensor("out"))


# revision 14
# speedup vs baseline: 1.5659x; 1.0248x over previous
"""Trainium2 Bass kernel for nn_AttnLayer (dense_transformer, sum-normalized attention).

Reference computation (per batch b, all fp32):
    d      = in_seq[:,b,:] @ W_in2enc.T + prev_target_seq[:,b,:] @ W_lab2enc.T + (b1+b2)
    S      = d @ E.T                      (E = enc_seq[:,b,:])
    ssum_l = sum_m S[l,m]                 (sum-normalization denominator)
    C      = S @ E
    out    = (C / ssum[:,None]) @ W_enc2in.T + b3

The attention is linear (sum-normalization, no softmax), so S is never
materialized: C = d @ (E^T E) = d @ G with the 512x512 Gram matrix G:
    G    = E-blocks.T @ E                            [e, e']  f32r
    H    = G-blocks.T @ W3T                          [e, o]   f32r  (= G @ W3T)
    d^T  = W1T.T @ X^T + W2T.T @ T^T + bd            [e, l]   f32r
    O    = d^T-blocks.T @ H                          [l, o]   f32r, then *1/ssum + b3

Denominator (exact fp32; ssum suffers catastrophic cancellation, min |ssum|
~0.05 vs ~700 typical, so this path must be fp32 end-to-end from raw inputs):
    ssum = X@v1 + T@v2 + esum.bd,  v1 = W1^T esum,  esum = sum_m E[m,:]
Everything on this path runs as matmuls whose MOVING operand is a [K,1]
column, which costs ~nothing on the PE (cost ~ moving rows only; stationary
loads are free):
  - esum[e]  = ones-column reduction of E chunks over partitions
  - v1,v2    = W^T @ esum-column (accumulated over e-chunks)
  - ssum[l]  = X-chunk.T @ v1-column (accumulated over chunks of both inputs)
HW-verified: fp32 ap-size-1 matmuls are exact-fp32-class; f32r data is
rounded to an 11-bit mantissa AT LOAD (DMA) or at engine write, NOT at PE
read.  So the fp32 inputs are loaded once exactly, the denominator reads
them directly, and cheap engine round-copies produce the f32r versions the
numerator matmuls consume (errors there stay relative to the numerator and
cancel against 1/ssum).

Sharding: data-parallel over batch B=16 across 8 cores (2 batches per core).
"""

import os

os.environ.setdefault("MYCRO_LOCAL_CACHE", "1")

import numpy as np

import concourse.bass as bass
from concourse import bacc
import concourse.mybir as mybir
import concourse.tile as tile
from concourse.bass_utils import run_bass_kernel_spmd

# Problem shape (hardcoded per contract)
L = 1024      # L_in == L_enc
B = 16
D = 512       # D_in == D_enc == D_emb
N_CORES = 8
BPC = B // N_CORES   # batches per core
P = 128
NE = D // P          # 4 chunks of contraction axes of size 512
NM = L // P          # 8 chunks of the L_enc axis
NL = L // P          # 8 chunks of the L_in axis
NLH = 2              # l processed in halves of 512 (moving-operand max for 4-byte)
LH = L // NLH

F32 = mybir.dt.float32
F32R = mybir.dt.float32r

# smallT PSUM column map: [0:32) esum partials (mc,ec), [32:36) v1,
# [36:40) v2, [40] c0, [41:49) ssum
C_ESUM = 0
C_V1 = 32
C_V2 = 36
C_C0 = 40
C_SS = 41


def build_nc():
    nc = bacc.Bacc(None, target_bir_lowering=False, debug=False)

    xT_d = nc.declare_dram_parameter("xT", [BPC, D, L], F32, isOutput=False)
    tT_d = nc.declare_dram_parameter("tT", [BPC, D, L], F32, isOutput=False)
    en_d = nc.declare_dram_parameter("en", [BPC, L, D], F32, isOutput=False)
    w1t_d = nc.declare_dram_parameter("w1t", [D, D], F32R, isOutput=False)  # [i, e]
    w2t_d = nc.declare_dram_parameter("w2t", [D, D], F32R, isOutput=False)  # [j, e]
    w3t_d = nc.declare_dram_parameter("w3t", [D, D], F32R, isOutput=False)  # [e, o]
    w1n_d = nc.declare_dram_parameter("w1n", [D, D], F32, isOutput=False)  # [e, i]
    w2n_d = nc.declare_dram_parameter("w2n", [D, D], F32, isOutput=False)  # [e, j]
    bd_d = nc.declare_dram_parameter("bd", [D], F32, isOutput=False)       # b1 + b2
    b3bc_d = nc.declare_dram_parameter("b3bc", [P, D], F32, isOutput=False)
    ones_d = nc.declare_dram_parameter("ones", [P, 1], F32, isOutput=False)
    out_d = nc.declare_dram_parameter("out", [BPC, L, D], F32, isOutput=True)

    AF = mybir.ActivationFunctionType
    AX = mybir.AxisListType

    with tile.TileContext(nc) as tc:
        with (
            tc.tile_pool(name="wpool", bufs=1) as wpool,
            tc.tile_pool(name="big", bufs=1) as big,
            tc.tile_pool(name="vecs", bufs=1) as vecs,
            tc.tile_pool(name="opool", bufs=10) as opool,
            tc.tile_pool(name="psA", bufs=4, space="PSUM") as psA,
            tc.tile_pool(name="psS", bufs=1, space="PSUM") as psS,
        ):
            # ---- persistent weights / constants ----
            w1t = wpool.tile([P, NE, D], F32R, name="w1t")   # [i%128, i//128, e]
            w2t = wpool.tile([P, NE, D], F32R, name="w2t")
            w3t = wpool.tile([P, NE, D], F32R, name="w3t")   # [e%128, e//128, o]
            w1n = wpool.tile([P, NE, D], F32, name="w1n")    # [e%128, e//128, i]
            w2n = wpool.tile([P, NE, D], F32, name="w2n")
            bd_sb = wpool.tile([P, NE], F32, name="bd_sb")
            b3bc = wpool.tile([P, D], F32, name="b3bc")
            ones = wpool.tile([P, 1], F32, name="ones")

            # Small constants on the ACT queue (tiny transfers).
            nc.scalar.dma_start(out=ones, in_=ones_d[:, :])
            nc.scalar.dma_start(
                out=bd_sb, in_=bd_d.rearrange("(c p) -> p c", p=P))
            nc.scalar.dma_start(out=b3bc, in_=b3bc_d[:, :])

            # ---- the single ordered bulk-load queue (SP): need order ----
            en = [None, None]
            xT = [None, None]
            tT = [None, None]
            en[0] = big.tile([P, NM, D], F32, name="en0", tag="slot_en")
            xT[0] = big.tile([P, NE, L], F32, name="xT0", tag="slot_x")
            tT[0] = big.tile([P, NE, L], F32, name="tT0", tag="slot_t")
            for mc in range(NM):
                nc.sync.dma_start(
                    out=en[0][:, mc, :], in_=en_d[0, mc * P : (mc + 1) * P, :])
            for k in range(NE):
                nc.sync.dma_start(
                    out=w1t[:, k, :], in_=w1t_d[k * P : (k + 1) * P, :])
                nc.sync.dma_start(
                    out=w2t[:, k, :], in_=w2t_d[k * P : (k + 1) * P, :])
                nc.sync.dma_start(out=xT[0][:, k, :], in_=xT_d[0, k * P : (k + 1) * P, :])
                nc.sync.dma_start(out=tT[0][:, k, :], in_=tT_d[0, k * P : (k + 1) * P, :])
            nc.sync.dma_start(
                out=w3t, in_=w3t_d.rearrange("(c p) e -> p c e", p=P))
            nc.sync.dma_start(
                out=w1n, in_=w1n_d.rearrange("(c p) e -> p c e", p=P))
            nc.sync.dma_start(
                out=w2n, in_=w2n_d.rearrange("(c p) e -> p c e", p=P))
            en[1] = big.tile([P, NM, D], F32, name="en1", tag="slot_en")
            for mc in range(NM):
                nc.sync.dma_start(
                    out=en[1][:, mc, :], in_=en_d[1, mc * P : (mc + 1) * P, :])
            xT[1] = big.tile([P, NE, L], F32, name="xT1", tag="slot_x")
            tT[1] = big.tile([P, NE, L], F32, name="tT1", tag="slot_t")
            for k in range(NE):
                nc.sync.dma_start(out=xT[1][:, k, :], in_=xT_d[1, k * P : (k + 1) * P, :])
                nc.sync.dma_start(out=tT[1][:, k, :], in_=tT_d[1, k * P : (k + 1) * P, :])

            # round-copy engine rotation (writes F32R <- reads F32, rounds)
            rr_eng = [nc.gpsimd, nc.vector, nc.scalar]

            def rcopy(i, out, in_):
                eng = rr_eng[i % 3]
                if eng is nc.scalar:
                    eng.activation(out, in_, AF.Copy, bias=0.0)
                else:
                    eng.tensor_copy(out, in_)

            for b in range(BPC):
                enb, xTb, tTb = en[b], xT[b], tT[b]

                smallT = psS.tile([P, C_SS + NL], F32, name="smallT", tag="smallps")

                # ---- per-chunk: esum partials (exact fp32 ap1s, single-
                # matmul groups), engine round-copy to en_r, then the chunk's
                # 4 Gram matmuls (mc-outer, 4-bank interleaved accumulation)
                # so PE consumes chunks at DMA-arrival rate ----
                en_r = big.tile([P, NM, D], F32R, name="en_r", tag="slot_enr")
                G_sb = big.tile([P, NE, D], F32R, name="G_sb", tag="slot_G")
                g_ps = [psA.tile([P, D], F32, name=f"g_ps{gc}", tag="acc")
                        for gc in range(NE)]
                for mc in range(NM):
                    for ec in range(NE):
                        nc.tensor.matmul(
                            smallT[:, C_ESUM + 4 * mc + ec : C_ESUM + 4 * mc + ec + 1],
                            enb[:, mc, ec * P : (ec + 1) * P],
                            ones,
                            start=True, stop=True,
                            skip_group_check=True,
                        )
                    rcopy(mc, en_r[:, mc, :], enb[:, mc, :])
                    for gc in range(NE):
                        nc.tensor.matmul(
                            g_ps[gc],
                            en_r[:, mc, gc * P : (gc + 1) * P],
                            en_r[:, mc, :],
                            start=(mc == 0), stop=(mc == NM - 1),
                        )
                for gc in range(NE):
                    nc.scalar.activation(G_sb[:, gc, :], g_ps[gc], AF.Copy, bias=0.0)

                # round-copies of X^T / T^T for the f32r dT matmuls; the exact
                # fp32 tiles keep serving the denominator below
                xT_r = big.tile([P, NE, L], F32R, name="xT_r", tag="slot_xr")
                tT_r = big.tile([P, NE, L], F32R, name="tT_r", tag="slot_tr")
                for k in range(NE):
                    rcopy(2 * k, xT_r[:, k, :], xTb[:, k, :])
                    rcopy(2 * k + 1, tT_r[:, k, :], tTb[:, k, :])

                # ---- d^T[e,l] = W1T.T @ X^T + W2T.T @ T^T + bd (f32r);
                # k-outer in DMA-arrival order (x0,t0,x1,t1,...) across 4
                # interleaved ec banks so PE paces with the loads ----
                dT = big.tile([P, NE, L], F32R, name="dT", tag="slot_d")
                K_ARRIVAL = [0, 4, 1, 5, 2, 6, 3, 7]
                for lh in range(NLH):
                    d_ps = [psA.tile([P, LH], F32, name=f"d_ps{ec}", tag="acc")
                            for ec in range(NE)]
                    for i, k in enumerate(K_ARRIVAL):
                        w = w1t if k < NE else w2t
                        src = xT_r if k < NE else tT_r
                        for ec in range(NE):
                            nc.tensor.matmul(
                                d_ps[ec],
                                w[:, k % NE, ec * P : (ec + 1) * P],
                                src[:, k % NE, lh * LH : (lh + 1) * LH],
                                start=(i == 0), stop=(i == 2 * NE - 1),
                            )
                    for ec in range(NE):
                        nc.vector.tensor_scalar_add(
                            dT[:, ec, lh * LH : (lh + 1) * LH], d_ps[ec],
                            bd_sb[:, ec : ec + 1],
                        )

                # ---- H[e,o] = G @ W3T (f32r); rcols compute on DVE meanwhile ----
                H_sb = big.tile([P, NE, D], F32R, name="H_sb", tag="slot_H")
                for hc in range(NE):
                    h_ps = psA.tile([P, D], F32, name="h_ps", tag="acc")
                    for kc in range(NE):
                        nc.tensor.matmul(
                            h_ps,
                            G_sb[:, kc, hc * P : (hc + 1) * P],
                            w3t[:, kc, :],
                            start=(kc == 0), stop=(kc == NE - 1),
                        )
                    nc.scalar.activation(H_sb[:, hc, :], h_ps, AF.Copy, bias=0.0)

                # esum consolidation on DVE: esum_sb[:, ec] = sum_mc partial
                esum_sb = vecs.tile([P, NE], F32, name="esum_sb")
                nc.vector.tensor_copy(esum_sb, smallT[:, C_ESUM : C_ESUM + 4])
                for mc in range(1, NM):
                    nc.vector.tensor_add(
                        esum_sb, esum_sb,
                        smallT[:, C_ESUM + 4 * mc : C_ESUM + 4 * mc + 4])
                # c0 partial: t[p] = sum_c bd[c,p]*esum[c,p]; bcast along free
                c0t = vecs.tile([P, 1], F32, name="c0t")
                c0m = vecs.tile([P, NE], F32, name="c0m")
                c0B = vecs.tile([P, P], F32, name="c0B")
                nc.vector.tensor_mul(c0m, bd_sb, esum_sb)
                nc.vector.reduce_sum(c0t, c0m, axis=AX.X)
                nc.vector.tensor_copy(c0B, c0t.broadcast_to([P, P]))

                # ---- v1 = W1^T esum, v2 = W2^T esum, c0 (ap1 fp32) ----
                for ic in range(NE):
                    for ec in range(NE):
                        nc.tensor.matmul(
                            smallT[:, C_V1 + ic : C_V1 + ic + 1],
                            w1n[:, ec, ic * P : (ic + 1) * P],
                            esum_sb[:, ec : ec + 1],
                            start=(ec == 0), stop=(ec == NE - 1),
                            skip_group_check=True,
                        )
                for ic in range(NE):
                    for ec in range(NE):
                        nc.tensor.matmul(
                            smallT[:, C_V2 + ic : C_V2 + ic + 1],
                            w2n[:, ec, ic * P : (ic + 1) * P],
                            esum_sb[:, ec : ec + 1],
                            start=(ec == 0), stop=(ec == NE - 1),
                            skip_group_check=True,
                        )
                nc.tensor.matmul(
                    smallT[:, C_C0 : C_C0 + 1], c0B, ones, start=True, stop=True,
                    skip_group_check=True,
                )
                v1c = vecs.tile([P, NE], F32, name="v1c")
                v2c = vecs.tile([P, NE], F32, name="v2c")
                c0c = vecs.tile([P, 1], F32, name="c0c")
                nc.scalar.activation(v1c, smallT[:, C_V1 : C_V1 + NE], AF.Copy, bias=0.0)
                nc.scalar.activation(v2c, smallT[:, C_V2 : C_V2 + NE], AF.Copy, bias=0.0)
                nc.scalar.activation(c0c, smallT[:, C_C0 : C_C0 + 1], AF.Copy, bias=0.0)

                # ---- ssum[l] = X@v1 + T@v2 directly as ap1 fp32 matmuls:
                # stationary = fp32 data chunk, moving = v column ----
                sc_sb = vecs.tile([P, NL], F32, name="sc_sb")
                rcols = vecs.tile([P, NL], F32, name="rcols")
                for lc in range(NL):
                    for k in range(2 * NE):
                        data = xTb if k < NE else tTb
                        vcol = v1c if k < NE else v2c
                        nc.tensor.matmul(
                            smallT[:, C_SS + lc : C_SS + lc + 1],
                            data[:, k % NE, lc * P : (lc + 1) * P],
                            vcol[:, k % NE : k % NE + 1],
                            start=(k == 0), stop=(k == 2 * NE - 1),
                            skip_group_check=True,
                        )
                for lc in range(NL):
                    nc.vector.tensor_scalar_add(
                        sc_sb[:, lc : lc + 1], smallT[:, C_SS + lc : C_SS + lc + 1],
                        c0c)
                    nc.vector.reciprocal(
                        rcols[:, lc : lc + 1], sc_sb[:, lc : lc + 1])

                # ---- output stage: O[l,o] = dT.T @ H, * 1/ssum, + b3 ----
                for lc in range(NL):
                    o_ps = psA.tile([P, D], F32, name="o_ps", tag="oacc", bufs=3)
                    for ec in range(NE):
                        nc.tensor.matmul(
                            o_ps,
                            dT[:, ec, lc * P : (lc + 1) * P],
                            H_sb[:, ec, :],
                            start=(ec == 0), stop=(ec == NE - 1),
                        )
                    o_sb = opool.tile([P, D], F32, name="o_sb")
                    nc.scalar.activation(
                        o_sb, o_ps, AF.Copy, bias=0.0,
                        scale=rcols[:, lc : lc + 1])
                    nc.gpsimd.tensor_add(o_sb, o_sb, b3bc)
                    nc.sync.dma_start(
                        out=out_d[b, lc * P : (lc + 1) * P, :], in_=o_sb)

    nc.finalize()
    return nc


def _make_in_maps(in_seq, enc_seq, prev_target_seq, W_in2enc, b_in2enc,
                  W_lab2enc, b_lab2enc, W_enc2in, b_enc2in):
    f32 = np.float32
    w1t = np.ascontiguousarray(np.asarray(W_in2enc, f32).T)   # [i, e]
    w2t = np.ascontiguousarray(np.asarray(W_lab2enc, f32).T)  # [j, e]
    w3t = np.ascontiguousarray(np.asarray(W_enc2in, f32).T)   # [e, o]
    w1n = np.ascontiguousarray(np.asarray(W_in2enc, f32))
    w2n = np.ascontiguousarray(np.asarray(W_lab2enc, f32))
    bd = np.ascontiguousarray(np.asarray(b_in2enc, f32) + np.asarray(b_lab2enc, f32))
    b3bc = np.ascontiguousarray(np.broadcast_to(np.asarray(b_enc2in, f32), (P, D)))
    ones = np.ones((P, 1), f32)

    in_maps = []
    for c in range(N_CORES):
        bs = slice(c * BPC, (c + 1) * BPC)
        x = np.asarray(in_seq[:, bs, :], f32)
        t = np.asarray(prev_target_seq[:, bs, :], f32)
        e = np.asarray(enc_seq[:, bs, :], f32)
        in_maps.append({
            "xT": np.ascontiguousarray(x.transpose(1, 2, 0)),
            "tT": np.ascontiguousarray(t.transpose(1, 2, 0)),
            "en": np.ascontiguousarray(e.transpose(1, 0, 2)),
            "w1t": w1t, "w2t": w2t, "w3t": w3t, "w1n": w1n, "w2n": w2n,
            "bd": bd, "b3bc": b3bc, "ones": ones,
        })
    return in_maps


_NC_CACHE = {}


def _get_nc():
    if "nc" not in _NC_CACHE:
        _NC_CACHE["nc"] = build_nc()
    return _NC_CACHE["nc"]


def kernel(**inputs):
    in_maps = _make_in_maps(**inputs)
    nc = _get_nc()
    res = run_bass_kernel_spmd(nc, in_maps, core_ids=list(range(N_CORES)))
    out = np.empty((L, B, D), np.float32)
    for c in range(N_CORES):
        per_core = res.results[c]["out"]  # (BPC, L, D)
        for j in range(BPC):
            out[:, c * BPC + j, :] = per_core[j]
    return out


def kernel_sim(core_id=0, **inputs):
    """CoreSim validation path: simulate one core, return its (BPC, L, D) output."""
    from concourse.bass_interp import CoreSim

    in_maps = _make_in_maps(**inputs)
    nc = _get_nc()
    sim = CoreSim(nc)
    for name, val in in_maps[core_id].items():
        sim.tensor(name)[:] = val
    sim.simulate(check_with_hw=False)
    return np.array(sim.tensor("out"))


# revision 17
# speedup vs baseline: 1.6620x; 1.0614x over previous
"""Trainium2 Bass kernel for nn_AttnLayer (dense_transformer, sum-normalized attention).

Reference computation (per batch b, all fp32):
    d      = in_seq[:,b,:] @ W_in2enc.T + prev_target_seq[:,b,:] @ W_lab2enc.T + (b1+b2)
    S      = d @ E.T                      (E = enc_seq[:,b,:])
    ssum_l = sum_m S[l,m]                 (sum-normalization denominator)
    C      = S @ E
    out    = (C / ssum[:,None]) @ W_enc2in.T + b3

The attention is linear (sum-normalization, no softmax), so S is never
materialized: C = d @ (E^T E) = d @ G with the 512x512 Gram matrix G:
    G    = E-blocks.T @ E                            [e, e']  f32r
    H    = G-blocks.T @ W3T                          [e, o]   f32r  (= G @ W3T)
    d^T  = W1T.T @ X^T + W2T.T @ T^T + bd            [e, l]   f32r
    O    = d^T-blocks.T @ H                          [l, o]   f32r, then *1/ssum + b3

Denominator (exact fp32; ssum suffers catastrophic cancellation, min |ssum|
~0.05 vs ~700 typical, so this path must be fp32 end-to-end from raw inputs):
    ssum = X@v1 + T@v2 + esum.bd,  v1 = W1^T esum,  esum = sum_m E[m,:]
Everything on this path runs as matmuls whose MOVING operand is a [K,1]
column, which costs ~nothing on the PE (cost ~ moving rows only; stationary
loads are free):
  - esum[e]  = ones-column reduction of E chunks over partitions
  - v1,v2    = W^T @ esum-column (accumulated over e-chunks)
  - ssum[l]  = X-chunk.T @ v1-column (accumulated over chunks of both inputs)
HW-verified: fp32 ap-size-1 matmuls are exact-fp32-class; f32r data is
rounded to an 11-bit mantissa AT LOAD (DMA) or at engine write, NOT at PE
read.  So the fp32 inputs are loaded once exactly, the denominator reads
them directly, and cheap engine round-copies produce the f32r versions the
numerator matmuls consume (errors there stay relative to the numerator and
cancel against 1/ssum).

Sharding: data-parallel over batch B=16 across 8 cores (2 batches per core).
"""

import os

os.environ.setdefault("MYCRO_LOCAL_CACHE", "1")

import numpy as np

import concourse.bass as bass
from concourse import bacc
import concourse.mybir as mybir
import concourse.tile as tile
from concourse.bass_utils import run_bass_kernel_spmd

# Problem shape (hardcoded per contract)
L = 1024      # L_in == L_enc
B = 16
D = 512       # D_in == D_enc == D_emb
N_CORES = 8
BPC = B // N_CORES   # batches per core
P = 128
NE = D // P          # 4 chunks of contraction axes of size 512
NM = L // P          # 8 chunks of the L_enc axis
NL = L // P          # 8 chunks of the L_in axis
NLH = 2              # l processed in halves of 512 (moving-operand max for 4-byte)
LH = L // NLH

F32 = mybir.dt.float32
F32R = mybir.dt.float32r

# smallT PSUM column map: [0:32) esum partials (mc,ec), [32:36) v1,
# [36:40) v2, [40] c0, [41:49) ssum
C_ESUM = 0
C_V1 = 32
C_V2 = 36
C_C0 = 40
C_SS = 41


def build_nc():
    nc = bacc.Bacc(None, target_bir_lowering=False, debug=False)

    xT_d = nc.declare_dram_parameter("xT", [BPC, D, L], F32, isOutput=False)
    tT_d = nc.declare_dram_parameter("tT", [BPC, D, L], F32, isOutput=False)
    en_d = nc.declare_dram_parameter("en", [BPC, L, D], F32, isOutput=False)
    w1t_d = nc.declare_dram_parameter("w1t", [D, D], F32R, isOutput=False)  # [i, e]
    w2t_d = nc.declare_dram_parameter("w2t", [D, D], F32R, isOutput=False)  # [j, e]
    w3t_d = nc.declare_dram_parameter("w3t", [D, D], F32R, isOutput=False)  # [e, o]
    w1n_d = nc.declare_dram_parameter("w1n", [D, D], F32, isOutput=False)  # [e, i]
    w2n_d = nc.declare_dram_parameter("w2n", [D, D], F32, isOutput=False)  # [e, j]
    bd_d = nc.declare_dram_parameter("bd", [D], F32, isOutput=False)       # b1 + b2
    b3bc_d = nc.declare_dram_parameter("b3bc", [P, D], F32, isOutput=False)
    ones_d = nc.declare_dram_parameter("ones", [P, 1], F32, isOutput=False)
    out_d = nc.declare_dram_parameter("out", [BPC, L, D], F32, isOutput=True)

    AF = mybir.ActivationFunctionType
    AX = mybir.AxisListType

    with tile.TileContext(nc) as tc:
        with (
            tc.tile_pool(name="wpool", bufs=1) as wpool,
            tc.tile_pool(name="big", bufs=1) as big,
            tc.tile_pool(name="vecs", bufs=1) as vecs,
            tc.tile_pool(name="opool", bufs=4) as opool,
            tc.tile_pool(name="psA", bufs=7, space="PSUM") as psA,
            tc.tile_pool(name="psS", bufs=1, space="PSUM") as psS,
        ):
            # ---- persistent weights / constants ----
            w1t = wpool.tile([P, NE, D], F32R, name="w1t")   # [i%128, i//128, e]
            w2t = wpool.tile([P, NE, D], F32R, name="w2t")
            w3t = wpool.tile([P, NE, D], F32R, name="w3t")   # [e%128, e//128, o]
            w1n = wpool.tile([P, NE, D], F32, name="w1n")    # [e%128, e//128, i]
            w2n = wpool.tile([P, NE, D], F32, name="w2n")
            bd_sb = wpool.tile([P, NE], F32, name="bd_sb")
            b3bc = wpool.tile([P, D], F32, name="b3bc")
            ones = wpool.tile([P, 1], F32, name="ones")

            # Small constants on the ACT queue (tiny transfers).
            nc.scalar.dma_start(out=ones, in_=ones_d[:, :])
            nc.scalar.dma_start(
                out=bd_sb, in_=bd_d.rearrange("(c p) -> p c", p=P))
            nc.scalar.dma_start(out=b3bc, in_=b3bc_d[:, :])

            # ---- single ordered bulk-load queue (SP), in need order.
            # Slot plan (16 KiB each, reused across lifetimes):
            #   slot_en : en0 F32   -> tT1 F32 (en0 dies after esum0+rounds)
            #   slot_en1: en1 F32 (own slot)
            #   slot_enr: en0_r -> en1_r -> xT1 F32 (Gram is the last reader)
            #   slot_x/t: xT0/tT0 F32 (die after ss0)
            #   slot_xr/tr: xT0_r/tT0_r -> xT1_r/tT1_r
            en0 = big.tile([P, NM, D], F32, name="en0", tag="slot_en")
            en1 = big.tile([P, NM, D], F32, name="en1", tag="slot_en1")
            xT0 = big.tile([P, NE, L], F32, name="xT0", tag="slot_x")
            tT0 = big.tile([P, NE, L], F32, name="tT0", tag="slot_t")
            for mc in range(NM):
                nc.sync.dma_start(
                    out=en0[:, mc, :], in_=en_d[0, mc * P : (mc + 1) * P, :])
            for mc in range(NM):
                nc.sync.dma_start(
                    out=en1[:, mc, :], in_=en_d[1, mc * P : (mc + 1) * P, :])
            for k in range(NE):
                nc.sync.dma_start(
                    out=w1t[:, k, :], in_=w1t_d[k * P : (k + 1) * P, :])
                nc.sync.dma_start(
                    out=w2t[:, k, :], in_=w2t_d[k * P : (k + 1) * P, :])
                nc.sync.dma_start(out=xT0[:, k, :], in_=xT_d[0, k * P : (k + 1) * P, :])
                nc.sync.dma_start(out=tT0[:, k, :], in_=tT_d[0, k * P : (k + 1) * P, :])
            nc.sync.dma_start(
                out=w3t, in_=w3t_d.rearrange("(c p) e -> p c e", p=P))
            nc.sync.dma_start(
                out=w1n, in_=w1n_d.rearrange("(c p) e -> p c e", p=P))
            nc.sync.dma_start(
                out=w2n, in_=w2n_d.rearrange("(c p) e -> p c e", p=P))
            en_r = [
                big.tile([P, NM, D], F32R, name="en_r0", tag="slot_enr"),
                big.tile([P, NM, D], F32R, name="en_r1", tag="slot_enr"),
            ]
            en = [en0, en1]
            xT = [xT0, None]
            tT = [tT0, None]

            # one PSUM bank of packed column accumulators for BOTH batches:
            # per batch base b*49: +0:32 esum partials, +32:36 v1, +36:40 v2,
            # +40 c0, +41:49 ssum
            smallT = psS.tile([P, 2 * 49], F32, name="smallT")

            def sT(b, c):
                return smallT[:, 49 * b + c : 49 * b + c + 1]

            # ---- phase helpers (engine roles: Pool = round-copies only,
            # DVE = adds/rcols/consolidation, ACT = psum copies + scales) ----
            G_sb = [None, None]
            dT_t = [None, None]
            H_sb = [None, None]
            xr_t = [None, None]
            tr_t = [None, None]
            esum_sb = [None, None]
            v1c = [None, None]
            v2c = [None, None]
            c0c = [None, None]
            rcols = [None, None]

            def phase_gram(b):
                # per-chunk: 4 esum-partial ap1s (fp32), Pool round-copy,
                # then the chunk's 4 Gram matmuls (mc-outer, 4-bank ILP)
                enb = en[b]
                G_sb[b] = big.tile([P, NE, D], F32R, name=f"G_sb{b}", tag=f"slot_G{b}")
                g_ps = [psA.tile([P, D], F32, name=f"g_ps{b}{gc}", tag="acc")
                        for gc in range(NE)]
                for mc in range(NM):
                    for ec in range(NE):
                        nc.tensor.matmul(
                            sT(b, C_ESUM + 4 * mc + ec),
                            enb[:, mc, ec * P : (ec + 1) * P],
                            ones,
                            start=True, stop=True,
                            skip_group_check=True,
                        )
                    nc.gpsimd.tensor_copy(en_r[b][:, mc, :], enb[:, mc, :])
                    for gc in range(NE):
                        nc.tensor.matmul(
                            g_ps[gc],
                            en_r[b][:, mc, gc * P : (gc + 1) * P],
                            en_r[b][:, mc, :],
                            start=(mc == 0), stop=(mc == NM - 1),
                        )
                for gc in range(NE):
                    nc.scalar.activation(
                        G_sb[b][:, gc, :], g_ps[gc], AF.Copy, bias=0.0)

            def phase_dT(b):
                # Pool round-copies in DMA-arrival order, then d^T k-outer in
                # arrival order (x0,t0,x1,t1,...) across 4 interleaved banks
                xr_t[b] = big.tile([P, NE, L], F32R, name=f"xT_r{b}", tag="slot_xr")
                tr_t[b] = big.tile([P, NE, L], F32R, name=f"tT_r{b}", tag="slot_tr")
                for k in range(NE):
                    nc.gpsimd.tensor_copy(xr_t[b][:, k, :], xT[b][:, k, :])
                    nc.gpsimd.tensor_copy(tr_t[b][:, k, :], tT[b][:, k, :])
                dT_t[b] = big.tile([P, NE, L], F32R, name=f"dT{b}", tag="slot_d")
                K_ARRIVAL = [0, 4, 1, 5, 2, 6, 3, 7]
                for lh in range(NLH):
                    d_ps = [psA.tile([P, LH], F32, name=f"d_ps{b}{ec}", tag="acc")
                            for ec in range(NE)]
                    for i, k in enumerate(K_ARRIVAL):
                        w = w1t if k < NE else w2t
                        src = xr_t[b] if k < NE else tr_t[b]
                        for ec in range(NE):
                            nc.tensor.matmul(
                                d_ps[ec],
                                w[:, k % NE, ec * P : (ec + 1) * P],
                                src[:, k % NE, lh * LH : (lh + 1) * LH],
                                start=(i == 0), stop=(i == 2 * NE - 1),
                            )
                    for ec in range(NE):
                        nc.vector.tensor_scalar_add(
                            dT_t[b][:, ec, lh * LH : (lh + 1) * LH], d_ps[ec],
                            bd_sb[:, ec : ec + 1],
                        )

            def phase_H(b):
                H_sb[b] = big.tile([P, NE, D], F32R, name=f"H_sb{b}", tag="slot_H")
                for hc in range(NE):
                    h_ps = psA.tile([P, D], F32, name=f"h_ps{b}", tag="acc")
                    for kc in range(NE):
                        nc.tensor.matmul(
                            h_ps,
                            G_sb[b][:, kc, hc * P : (hc + 1) * P],
                            w3t[:, kc, :],
                            start=(kc == 0), stop=(kc == NE - 1),
                        )
                    nc.scalar.activation(
                        H_sb[b][:, hc, :], h_ps, AF.Copy, bias=0.0)

            def phase_denom(b):
                # DVE esum consolidation, c0 prep, PE v/c0 ap1s, ACT col
                # copies, PE ssum ap1s, DVE rcols
                esum_sb[b] = vecs.tile([P, NE], F32, name=f"esum_sb{b}")
                nc.vector.tensor_copy(
                    esum_sb[b], smallT[:, 49 * b + C_ESUM : 49 * b + C_ESUM + 4])
                for mc in range(1, NM):
                    nc.vector.tensor_add(
                        esum_sb[b], esum_sb[b],
                        smallT[:, 49 * b + C_ESUM + 4 * mc
                               : 49 * b + C_ESUM + 4 * mc + 4])
                c0t = vecs.tile([P, 1], F32, name=f"c0t{b}")
                c0m = vecs.tile([P, NE], F32, name=f"c0m{b}")
                c0B = vecs.tile([P, P], F32, name=f"c0B{b}")
                nc.vector.tensor_mul(c0m, bd_sb, esum_sb[b])
                nc.vector.reduce_sum(c0t, c0m, axis=AX.X)
                nc.vector.tensor_copy(c0B, c0t.broadcast_to([P, P]))
                for ic in range(NE):
                    for ec in range(NE):
                        nc.tensor.matmul(
                            sT(b, C_V1 + ic),
                            w1n[:, ec, ic * P : (ic + 1) * P],
                            esum_sb[b][:, ec : ec + 1],
                            start=(ec == 0), stop=(ec == NE - 1),
                            skip_group_check=True,
                        )
                for ic in range(NE):
                    for ec in range(NE):
                        nc.tensor.matmul(
                            sT(b, C_V2 + ic),
                            w2n[:, ec, ic * P : (ic + 1) * P],
                            esum_sb[b][:, ec : ec + 1],
                            start=(ec == 0), stop=(ec == NE - 1),
                            skip_group_check=True,
                        )
                nc.tensor.matmul(
                    sT(b, C_C0), c0B, ones, start=True, stop=True,
                    skip_group_check=True,
                )
                v1c[b] = vecs.tile([P, NE], F32, name=f"v1c{b}")
                v2c[b] = vecs.tile([P, NE], F32, name=f"v2c{b}")
                c0c[b] = vecs.tile([P, 1], F32, name=f"c0c{b}")
                nc.scalar.activation(
                    v1c[b], smallT[:, 49 * b + C_V1 : 49 * b + C_V1 + NE],
                    AF.Copy, bias=0.0)
                nc.scalar.activation(
                    v2c[b], smallT[:, 49 * b + C_V2 : 49 * b + C_V2 + NE],
                    AF.Copy, bias=0.0)
                nc.scalar.activation(
                    c0c[b], smallT[:, 49 * b + C_C0 : 49 * b + C_C0 + 1],
                    AF.Copy, bias=0.0)
                sc_sb = vecs.tile([P, NL], F32, name=f"sc_sb{b}")
                rcols[b] = vecs.tile([P, NL], F32, name=f"rcols{b}")
                for lc in range(NL):
                    for k in range(2 * NE):
                        data = xT[b] if k < NE else tT[b]
                        vcol = v1c[b] if k < NE else v2c[b]
                        nc.tensor.matmul(
                            sT(b, C_SS + lc),
                            data[:, k % NE, lc * P : (lc + 1) * P],
                            vcol[:, k % NE : k % NE + 1],
                            start=(k == 0), stop=(k == 2 * NE - 1),
                            skip_group_check=True,
                        )
                for lc in range(NL):
                    nc.vector.tensor_scalar_add(
                        sc_sb[:, lc : lc + 1], sT(b, C_SS + lc), c0c[b])
                    nc.vector.reciprocal(
                        rcols[b][:, lc : lc + 1], sc_sb[:, lc : lc + 1])

            def phase_O(b):
                for lc in range(NL):
                    o_ps = psA.tile([P, D], F32, name=f"o_ps{b}", tag="acc")
                    for ec in range(NE):
                        nc.tensor.matmul(
                            o_ps,
                            dT_t[b][:, ec, lc * P : (lc + 1) * P],
                            H_sb[b][:, ec, :],
                            start=(ec == 0), stop=(ec == NE - 1),
                        )
                    o_sb = opool.tile([P, D], F32, name="o_sb")
                    nc.scalar.activation(
                        o_sb, o_ps, AF.Copy, bias=0.0,
                        scale=rcols[b][:, lc : lc + 1])
                    nc.vector.tensor_add(o_sb, o_sb, b3bc)
                    nc.sync.dma_start(
                        out=out_d[b, lc * P : (lc + 1) * P, :], in_=o_sb)

            # ---- global PE schedule: both Grams first (batch-1 Gram fills
            # batch-0's load window), then per-batch pipelines ----
            phase_gram(0)
            phase_gram(1)
            # batch-1 inputs reuse slot_enr/slot_en; the dmas are emitted
            # here so the tag-ring (emission) order matches the lifetimes:
            # en_r0 -> en_r1 -> xT1, and en0 -> tT1.
            xT[1] = big.tile([P, NE, L], F32, name="xT1", tag="slot_enr")
            tT[1] = big.tile([P, NE, L], F32, name="tT1", tag="slot_en")
            for k in range(NE):
                nc.sync.dma_start(out=xT[1][:, k, :], in_=xT_d[1, k * P : (k + 1) * P, :])
                nc.sync.dma_start(out=tT[1][:, k, :], in_=tT_d[1, k * P : (k + 1) * P, :])
            phase_dT(0)
            phase_H(0)
            phase_denom(0)
            phase_O(0)
            phase_dT(1)
            phase_H(1)
            phase_denom(1)
            phase_O(1)

    nc.finalize()
    return nc


def _make_in_maps(in_seq, enc_seq, prev_target_seq, W_in2enc, b_in2enc,
                  W_lab2enc, b_lab2enc, W_enc2in, b_enc2in):
    f32 = np.float32
    w1t = np.ascontiguousarray(np.asarray(W_in2enc, f32).T)   # [i, e]
    w2t = np.ascontiguousarray(np.asarray(W_lab2enc, f32).T)  # [j, e]
    w3t = np.ascontiguousarray(np.asarray(W_enc2in, f32).T)   # [e, o]
    w1n = np.ascontiguousarray(np.asarray(W_in2enc, f32))
    w2n = np.ascontiguousarray(np.asarray(W_lab2enc, f32))
    bd = np.ascontiguousarray(np.asarray(b_in2enc, f32) + np.asarray(b_lab2enc, f32))
    b3bc = np.ascontiguousarray(np.broadcast_to(np.asarray(b_enc2in, f32), (P, D)))
    ones = np.ones((P, 1), f32)

    in_maps = []
    for c in range(N_CORES):
        bs = slice(c * BPC, (c + 1) * BPC)
        x = np.asarray(in_seq[:, bs, :], f32)
        t = np.asarray(prev_target_seq[:, bs, :], f32)
        e = np.asarray(enc_seq[:, bs, :], f32)
        in_maps.append({
            "xT": np.ascontiguousarray(x.transpose(1, 2, 0)),
            "tT": np.ascontiguousarray(t.transpose(1, 2, 0)),
            "en": np.ascontiguousarray(e.transpose(1, 0, 2)),
            "w1t": w1t, "w2t": w2t, "w3t": w3t, "w1n": w1n, "w2n": w2n,
            "bd": bd, "b3bc": b3bc, "ones": ones,
        })
    return in_maps


_NC_CACHE = {}


def _get_nc():
    if "nc" not in _NC_CACHE:
        _NC_CACHE["nc"] = build_nc()
    return _NC_CACHE["nc"]


def kernel(**inputs):
    in_maps = _make_in_maps(**inputs)
    nc = _get_nc()
    res = run_bass_kernel_spmd(nc, in_maps, core_ids=list(range(N_CORES)))
    out = np.empty((L, B, D), np.float32)
    for c in range(N_CORES):
        per_core = res.results[c]["out"]  # (BPC, L, D)
        for j in range(BPC):
            out[:, c * BPC + j, :] = per_core[j]
    return out


def kernel_sim(core_id=0, **inputs):
    """CoreSim validation path: simulate one core, return its (BPC, L, D) output."""
    from concourse.bass_interp import CoreSim

    in_maps = _make_in_maps(**inputs)
    nc = _get_nc()
    sim = CoreSim(nc)
    for name, val in in_maps[core_id].items():
        sim.tensor(name)[:] = val
    sim.simulate(check_with_hw=False)
    return np.array(sim.tensor("out"))


# revision 18
# speedup vs baseline: 1.6953x; 1.0200x over previous
"""Trainium2 Bass kernel for nn_AttnLayer (dense_transformer, sum-normalized attention).

Reference computation (per batch b, all fp32):
    d      = in_seq[:,b,:] @ W_in2enc.T + prev_target_seq[:,b,:] @ W_lab2enc.T + (b1+b2)
    S      = d @ E.T                      (E = enc_seq[:,b,:])
    ssum_l = sum_m S[l,m]                 (sum-normalization denominator)
    C      = S @ E
    out    = (C / ssum[:,None]) @ W_enc2in.T + b3

The attention is linear (sum-normalization, no softmax), so S is never
materialized: C = d @ (E^T E) = d @ G with the 512x512 Gram matrix G:
    G    = E-blocks.T @ E                            [e, e']  f32r
    H    = G-blocks.T @ W3T                          [e, o]   f32r  (= G @ W3T)
    d^T  = W1T.T @ X^T + W2T.T @ T^T + bd            [e, l]   f32r
    O    = d^T-blocks.T @ H                          [l, o]   f32r, then *1/ssum + b3

Denominator (exact fp32; ssum suffers catastrophic cancellation, min |ssum|
~0.05 vs ~700 typical, so this path must be fp32 end-to-end from raw inputs):
    ssum = X@v1 + T@v2 + esum.bd,  v1 = W1^T esum,  esum = sum_m E[m,:]
Everything on this path runs as matmuls whose MOVING operand is a [K,1]
column, which costs ~nothing on the PE (cost ~ moving rows only; stationary
loads are free):
  - esum[e]  = ones-column reduction of E chunks over partitions
  - v1,v2    = W^T @ esum-column (accumulated over e-chunks)
  - ssum[l]  = X-chunk.T @ v1-column (accumulated over chunks of both inputs)
HW-verified: fp32 ap-size-1 matmuls are exact-fp32-class; f32r data is
rounded to an 11-bit mantissa AT LOAD (DMA) or at engine write, NOT at PE
read.  So the fp32 inputs are loaded once exactly, the denominator reads
them directly, and cheap engine round-copies produce the f32r versions the
numerator matmuls consume (errors there stay relative to the numerator and
cancel against 1/ssum).

Sharding: data-parallel over batch B=16 across 8 cores (2 batches per core).
"""

import os

os.environ.setdefault("MYCRO_LOCAL_CACHE", "1")

import numpy as np

import concourse.bass as bass
from concourse import bacc
import concourse.mybir as mybir
import concourse.tile as tile
from concourse.bass_utils import run_bass_kernel_spmd

# Problem shape (hardcoded per contract)
L = 1024      # L_in == L_enc
B = 16
D = 512       # D_in == D_enc == D_emb
N_CORES = 8
BPC = B // N_CORES   # batches per core
P = 128
NE = D // P          # 4 chunks of contraction axes of size 512
NM = L // P          # 8 chunks of the L_enc axis
NL = L // P          # 8 chunks of the L_in axis
NLH = 2              # l processed in halves of 512 (moving-operand max for 4-byte)
LH = L // NLH

F32 = mybir.dt.float32
F32R = mybir.dt.float32r

# smallT PSUM column map: [0:32) esum partials (mc,ec), [32:36) v1,
# [36:40) v2, [40] c0, [41:49) ssum
C_ESUM = 0
C_V1 = 32
C_V2 = 36
C_C0 = 40
C_SS = 41


def build_nc():
    nc = bacc.Bacc(None, target_bir_lowering=False, debug=False)

    xT_d = nc.declare_dram_parameter("xT", [BPC, D, L], F32, isOutput=False)
    tT_d = nc.declare_dram_parameter("tT", [BPC, D, L], F32, isOutput=False)
    en_d = nc.declare_dram_parameter("en", [BPC, L, D], F32, isOutput=False)
    w1t_d = nc.declare_dram_parameter("w1t", [D, D], F32R, isOutput=False)  # [i, e]
    w2t_d = nc.declare_dram_parameter("w2t", [D, D], F32R, isOutput=False)  # [j, e]
    w3t_d = nc.declare_dram_parameter("w3t", [D, D], F32R, isOutput=False)  # [e, o]
    w1n_d = nc.declare_dram_parameter("w1n", [D, D], F32, isOutput=False)  # [e, i]
    w2n_d = nc.declare_dram_parameter("w2n", [D, D], F32, isOutput=False)  # [e, j]
    bd_d = nc.declare_dram_parameter("bd", [D], F32, isOutput=False)       # b1 + b2
    b3bc_d = nc.declare_dram_parameter("b3bc", [P, D], F32, isOutput=False)
    ones_d = nc.declare_dram_parameter("ones", [P, 1], F32, isOutput=False)
    out_d = nc.declare_dram_parameter("out", [BPC, L, D], F32, isOutput=True)

    AF = mybir.ActivationFunctionType
    AX = mybir.AxisListType

    with tile.TileContext(nc) as tc:
        with (
            tc.tile_pool(name="wpool", bufs=1) as wpool,
            tc.tile_pool(name="big", bufs=1) as big,
            tc.tile_pool(name="vecs", bufs=1) as vecs,
            tc.tile_pool(name="opool", bufs=4) as opool,
            tc.tile_pool(name="psA", bufs=7, space="PSUM") as psA,
            tc.tile_pool(name="psS", bufs=1, space="PSUM") as psS,
        ):
            # ---- persistent weights / constants ----
            w1t = wpool.tile([P, NE, D], F32R, name="w1t")   # [i%128, i//128, e]
            w2t = wpool.tile([P, NE, D], F32R, name="w2t")
            w3t = wpool.tile([P, NE, D], F32R, name="w3t")   # [e%128, e//128, o]
            w1n = wpool.tile([P, NE, D], F32, name="w1n")    # [e%128, e//128, i]
            w2n = wpool.tile([P, NE, D], F32, name="w2n")
            bd_sb = wpool.tile([P, NE], F32, name="bd_sb")
            b3bc = wpool.tile([P, D], F32, name="b3bc")
            ones = wpool.tile([P, 1], F32, name="ones")

            # Small constants on the ACT queue (tiny transfers).
            nc.scalar.dma_start(out=ones, in_=ones_d[:, :])
            nc.scalar.dma_start(
                out=bd_sb, in_=bd_d.rearrange("(c p) -> p c", p=P))
            nc.scalar.dma_start(out=b3bc, in_=b3bc_d[:, :])

            # ---- single ordered bulk-load queue (SP), in need order.
            # Slot plan (16 KiB each, reused across lifetimes):
            #   slot_en : en0 F32   -> tT1 F32 (en0 dies after esum0+rounds)
            #   slot_en1: en1 F32 (own slot)
            #   slot_enr: en0_r -> en1_r -> xT1 F32 (Gram is the last reader)
            #   slot_x/t: xT0/tT0 F32 (die after ss0)
            #   slot_xr/tr: xT0_r/tT0_r -> xT1_r/tT1_r
            en0 = big.tile([P, NM, D], F32, name="en0", tag="slot_en")
            en1 = big.tile([P, NM, D], F32, name="en1", tag="slot_en1")
            xT0 = big.tile([P, NE, L], F32, name="xT0", tag="slot_x")
            tT0 = big.tile([P, NE, L], F32, name="tT0", tag="slot_t")
            for mc in range(NM):
                eng = nc.gpsimd if mc == 0 else nc.sync
                eng.dma_start(
                    out=en0[:, mc, :], in_=en_d[0, mc * P : (mc + 1) * P, :])
            for mc in range(NM):
                nc.sync.dma_start(
                    out=en1[:, mc, :], in_=en_d[1, mc * P : (mc + 1) * P, :])
            for k in range(NE):
                nc.sync.dma_start(
                    out=w1t[:, k, :], in_=w1t_d[k * P : (k + 1) * P, :])
                nc.sync.dma_start(
                    out=w2t[:, k, :], in_=w2t_d[k * P : (k + 1) * P, :])
                nc.sync.dma_start(out=xT0[:, k, :], in_=xT_d[0, k * P : (k + 1) * P, :])
                nc.sync.dma_start(out=tT0[:, k, :], in_=tT_d[0, k * P : (k + 1) * P, :])
            nc.sync.dma_start(
                out=w3t, in_=w3t_d.rearrange("(c p) e -> p c e", p=P))
            nc.sync.dma_start(
                out=w1n, in_=w1n_d.rearrange("(c p) e -> p c e", p=P))
            nc.sync.dma_start(
                out=w2n, in_=w2n_d.rearrange("(c p) e -> p c e", p=P))
            en_r = [
                big.tile([P, NM, D], F32R, name="en_r0", tag="slot_enr"),
                big.tile([P, NM, D], F32R, name="en_r1", tag="slot_enr"),
            ]
            en = [en0, en1]
            xT = [xT0, None]
            tT = [tT0, None]

            # one PSUM bank of packed column accumulators for BOTH batches:
            # per batch base b*49: +0:32 esum partials, +32:36 v1, +36:40 v2,
            # +40 c0, +41:49 ssum
            smallT = psS.tile([P, 2 * 49], F32, name="smallT")

            def sT(b, c):
                return smallT[:, 49 * b + c : 49 * b + c + 1]

            # ---- phase helpers (engine roles: Pool = round-copies only,
            # DVE = adds/rcols/consolidation, ACT = psum copies + scales) ----
            G_sb = [None, None]
            dT_t = [None, None]
            H_sb = [None, None]
            xr_t = [None, None]
            tr_t = [None, None]
            esum_sb = [None, None]
            v1c = [None, None]
            v2c = [None, None]
            c0c = [None, None]
            rcols = [None, None]

            def phase_gram(b):
                # per-chunk: 4 esum-partial ap1s (fp32), Pool round-copy,
                # then the chunk's 4 Gram matmuls (mc-outer, 4-bank ILP)
                enb = en[b]
                G_sb[b] = big.tile([P, NE, D], F32R, name=f"G_sb{b}", tag=f"slot_G{b}")
                g_ps = [psA.tile([P, D], F32, name=f"g_ps{b}{gc}", tag="acc")
                        for gc in range(NE)]
                for mc in range(NM):
                    for ec in range(NE):
                        nc.tensor.matmul(
                            sT(b, C_ESUM + 4 * mc + ec),
                            enb[:, mc, ec * P : (ec + 1) * P],
                            ones,
                            start=True, stop=True,
                            skip_group_check=True,
                        )
                    nc.gpsimd.tensor_copy(en_r[b][:, mc, :], enb[:, mc, :])
                    for gc in range(NE):
                        nc.tensor.matmul(
                            g_ps[gc],
                            en_r[b][:, mc, gc * P : (gc + 1) * P],
                            en_r[b][:, mc, :],
                            start=(mc == 0), stop=(mc == NM - 1),
                        )
                for gc in range(NE):
                    nc.scalar.activation(
                        G_sb[b][:, gc, :], g_ps[gc], AF.Copy, bias=0.0)

            def phase_dT(b):
                # Pool round-copies in DMA-arrival order, then d^T k-outer in
                # arrival order (x0,t0,x1,t1,...) across 4 interleaved banks
                xr_t[b] = big.tile([P, NE, L], F32R, name=f"xT_r{b}", tag="slot_xr")
                tr_t[b] = big.tile([P, NE, L], F32R, name=f"tT_r{b}", tag="slot_tr")
                for k in range(NE):
                    nc.gpsimd.tensor_copy(xr_t[b][:, k, :], xT[b][:, k, :])
                    nc.vector.tensor_copy(tr_t[b][:, k, :], tT[b][:, k, :])
                dT_t[b] = big.tile([P, NE, L], F32R, name=f"dT{b}", tag="slot_d")
                K_ARRIVAL = [0, 4, 1, 5, 2, 6, 3, 7]
                for lh in range(NLH):
                    d_ps = [psA.tile([P, LH], F32, name=f"d_ps{b}{ec}", tag="acc")
                            for ec in range(NE)]
                    for i, k in enumerate(K_ARRIVAL):
                        w = w1t if k < NE else w2t
                        src = xr_t[b] if k < NE else tr_t[b]
                        for ec in range(NE):
                            nc.tensor.matmul(
                                d_ps[ec],
                                w[:, k % NE, ec * P : (ec + 1) * P],
                                src[:, k % NE, lh * LH : (lh + 1) * LH],
                                start=(i == 0), stop=(i == 2 * NE - 1),
                            )
                    for ec in range(NE):
                        nc.vector.tensor_scalar_add(
                            dT_t[b][:, ec, lh * LH : (lh + 1) * LH], d_ps[ec],
                            bd_sb[:, ec : ec + 1],
                        )

            def phase_H(b):
                H_sb[b] = big.tile([P, NE, D], F32R, name=f"H_sb{b}", tag="slot_H")
                for hc in range(NE):
                    h_ps = psA.tile([P, D], F32, name=f"h_ps{b}", tag="acc")
                    for kc in range(NE):
                        nc.tensor.matmul(
                            h_ps,
                            G_sb[b][:, kc, hc * P : (hc + 1) * P],
                            w3t[:, kc, :],
                            start=(kc == 0), stop=(kc == NE - 1),
                        )
                    nc.scalar.activation(
                        H_sb[b][:, hc, :], h_ps, AF.Copy, bias=0.0)

            c0B = [None, None]

            def phase_denom_prep(b):
                # DVE esum consolidation + c0 prep (inputs ready right after
                # the batch's Gram/esum phase)
                esum_sb[b] = vecs.tile([P, NE], F32, name=f"esum_sb{b}")
                nc.vector.tensor_copy(
                    esum_sb[b], smallT[:, 49 * b + C_ESUM : 49 * b + C_ESUM + 4])
                for mc in range(1, NM):
                    nc.vector.tensor_add(
                        esum_sb[b], esum_sb[b],
                        smallT[:, 49 * b + C_ESUM + 4 * mc
                               : 49 * b + C_ESUM + 4 * mc + 4])
                c0t = vecs.tile([P, 1], F32, name=f"c0t{b}")
                c0m = vecs.tile([P, NE], F32, name=f"c0m{b}")
                c0B[b] = vecs.tile([P, P], F32, name=f"c0B{b}")
                nc.vector.tensor_mul(c0m, bd_sb, esum_sb[b])
                nc.vector.reduce_sum(c0t, c0m, axis=AX.X)
                nc.vector.tensor_copy(c0B[b], c0t.broadcast_to([P, P]))

            def phase_denom(b):
                # PE v/c0 ap1s, ACT col copies, PE ssum ap1s, DVE rcols
                for ic in range(NE):
                    for ec in range(NE):
                        nc.tensor.matmul(
                            sT(b, C_V1 + ic),
                            w1n[:, ec, ic * P : (ic + 1) * P],
                            esum_sb[b][:, ec : ec + 1],
                            start=(ec == 0), stop=(ec == NE - 1),
                            skip_group_check=True,
                        )
                for ic in range(NE):
                    for ec in range(NE):
                        nc.tensor.matmul(
                            sT(b, C_V2 + ic),
                            w2n[:, ec, ic * P : (ic + 1) * P],
                            esum_sb[b][:, ec : ec + 1],
                            start=(ec == 0), stop=(ec == NE - 1),
                            skip_group_check=True,
                        )
                nc.tensor.matmul(
                    sT(b, C_C0), c0B[b], ones, start=True, stop=True,
                    skip_group_check=True,
                )
                v1c[b] = vecs.tile([P, NE], F32, name=f"v1c{b}")
                v2c[b] = vecs.tile([P, NE], F32, name=f"v2c{b}")
                c0c[b] = vecs.tile([P, 1], F32, name=f"c0c{b}")
                nc.scalar.activation(
                    v1c[b], smallT[:, 49 * b + C_V1 : 49 * b + C_V1 + NE],
                    AF.Copy, bias=0.0)
                nc.scalar.activation(
                    v2c[b], smallT[:, 49 * b + C_V2 : 49 * b + C_V2 + NE],
                    AF.Copy, bias=0.0)
                nc.scalar.activation(
                    c0c[b], smallT[:, 49 * b + C_C0 : 49 * b + C_C0 + 1],
                    AF.Copy, bias=0.0)
                sc_sb = vecs.tile([P, NL], F32, name=f"sc_sb{b}")
                rcols[b] = vecs.tile([P, NL], F32, name=f"rcols{b}")
                for lc in range(NL):
                    for k in range(2 * NE):
                        data = xT[b] if k < NE else tT[b]
                        vcol = v1c[b] if k < NE else v2c[b]
                        nc.tensor.matmul(
                            sT(b, C_SS + lc),
                            data[:, k % NE, lc * P : (lc + 1) * P],
                            vcol[:, k % NE : k % NE + 1],
                            start=(k == 0), stop=(k == 2 * NE - 1),
                            skip_group_check=True,
                        )
                for lc in range(NL):
                    nc.vector.tensor_scalar_add(
                        sc_sb[:, lc : lc + 1], sT(b, C_SS + lc), c0c[b])
                    nc.vector.reciprocal(
                        rcols[b][:, lc : lc + 1], sc_sb[:, lc : lc + 1])

            def phase_O(b):
                for lc in range(NL):
                    o_ps = psA.tile([P, D], F32, name=f"o_ps{b}", tag="acc")
                    for ec in range(NE):
                        nc.tensor.matmul(
                            o_ps,
                            dT_t[b][:, ec, lc * P : (lc + 1) * P],
                            H_sb[b][:, ec, :],
                            start=(ec == 0), stop=(ec == NE - 1),
                        )
                    o_sb = opool.tile([P, D], F32, name="o_sb")
                    nc.scalar.activation(
                        o_sb, o_ps, AF.Copy, bias=0.0,
                        scale=rcols[b][:, lc : lc + 1])
                    nc.vector.tensor_add(o_sb, o_sb, b3bc)
                    nc.sync.dma_start(
                        out=out_d[b, lc * P : (lc + 1) * P, :], in_=o_sb)

            # ---- global PE schedule: both Grams first (batch-1 Gram fills
            # batch-0's load window), then per-batch pipelines ----
            phase_gram(0)
            phase_denom_prep(0)
            phase_gram(1)
            phase_denom_prep(1)
            # batch-1 inputs reuse slot_enr/slot_en; the dmas are emitted
            # here so the tag-ring (emission) order matches the lifetimes:
            # en_r0 -> en_r1 -> xT1, and en0 -> tT1.
            xT[1] = big.tile([P, NE, L], F32, name="xT1", tag="slot_enr")
            tT[1] = big.tile([P, NE, L], F32, name="tT1", tag="slot_en")
            for k in range(NE):
                nc.sync.dma_start(out=xT[1][:, k, :], in_=xT_d[1, k * P : (k + 1) * P, :])
                nc.sync.dma_start(out=tT[1][:, k, :], in_=tT_d[1, k * P : (k + 1) * P, :])
            phase_dT(0)
            phase_H(0)
            phase_denom(0)
            phase_O(0)
            phase_dT(1)
            phase_H(1)
            phase_denom(1)
            phase_O(1)

    nc.finalize()
    return nc


def _make_in_maps(in_seq, enc_seq, prev_target_seq, W_in2enc, b_in2enc,
                  W_lab2enc, b_lab2enc, W_enc2in, b_enc2in):
    f32 = np.float32
    w1t = np.ascontiguousarray(np.asarray(W_in2enc, f32).T)   # [i, e]
    w2t = np.ascontiguousarray(np.asarray(W_lab2enc, f32).T)  # [j, e]
    w3t = np.ascontiguousarray(np.asarray(W_enc2in, f32).T)   # [e, o]
    w1n = np.ascontiguousarray(np.asarray(W_in2enc, f32))
    w2n = np.ascontiguousarray(np.asarray(W_lab2enc, f32))
    bd = np.ascontiguousarray(np.asarray(b_in2enc, f32) + np.asarray(b_lab2enc, f32))
    b3bc = np.ascontiguousarray(np.broadcast_to(np.asarray(b_enc2in, f32), (P, D)))
    ones = np.ones((P, 1), f32)

    in_maps = []
    for c in range(N_CORES):
        bs = slice(c * BPC, (c + 1) * BPC)
        x = np.asarray(in_seq[:, bs, :], f32)
        t = np.asarray(prev_target_seq[:, bs, :], f32)
        e = np.asarray(enc_seq[:, bs, :], f32)
        in_maps.append({
            "xT": np.ascontiguousarray(x.transpose(1, 2, 0)),
            "tT": np.ascontiguousarray(t.transpose(1, 2, 0)),
            "en": np.ascontiguousarray(e.transpose(1, 0, 2)),
            "w1t": w1t, "w2t": w2t, "w3t": w3t, "w1n": w1n, "w2n": w2n,
            "bd": bd, "b3bc": b3bc, "ones": ones,
        })
    return in_maps


_NC_CACHE = {}


def _get_nc():
    if "nc" not in _NC_CACHE:
        _NC_CACHE["nc"] = build_nc()
    return _NC_CACHE["nc"]


def kernel(**inputs):
    in_maps = _make_in_maps(**inputs)
    nc = _get_nc()
    res = run_bass_kernel_spmd(nc, in_maps, core_ids=list(range(N_CORES)))
    out = np.empty((L, B, D), np.float32)
    for c in range(N_CORES):
        per_core = res.results[c]["out"]  # (BPC, L, D)
        for j in range(BPC):
            out[:, c * BPC + j, :] = per_core[j]
    return out


def kernel_sim(core_id=0, **inputs):
    """CoreSim validation path: simulate one core, return its (BPC, L, D) output."""
    from concourse.bass_interp import CoreSim

    in_maps = _make_in_maps(**inputs)
    nc = _get_nc()
    sim = CoreSim(nc)
    for name, val in in_maps[core_id].items():
        sim.tensor(name)[:] = val
    sim.simulate(check_with_hw=False)
    return np.array(sim.tensor("out"))
